# revision 32
# baseline (speedup 1.0000x reference)
"""Bass/Trainium2 kernel for batched masked-Kabsch RMSD (nn_Coords2RMSD).

PE-centric design, one program per distinct per-core shape (8 cores):
  - Host sorts rows by num_atoms into 32 tiles of 128 rows and
    snake-assigns 4 tiles per core (big-first). Per tile, coords are
    repacked TRANSPOSED into fp8e4m3: atoms on SBUF partitions; for each
    group of 16 rows a 112-column operand [x y z X Y Z 1] per row.
    Padding atoms are zeroed on the host; the ones column makes the Gram
    matrix carry the masked sums.
  - Per (group, pair-of-128-atom-chunks) ONE symmetric DoubleRow fp8
    matmul op^T @ op accumulates in PSUM: the diagonal 7x7 blocks hold
    all 21 per-row statistics (cross-covariance, |x|^2, |y|^2, sums).
    A warmup matmul burst pins the PE pstate ramp before the real work.
  - Extraction: Act copies PSUM->SBUF (bf16), 16 identity-select
    matmuls gather the diagonal slot blocks into a second PSUM, Act
    copies them to a staging buffer, and 7 strided DMAs per tile-pair
    transpose [slot-comp, row] -> [row, stats] (sums row first so the
    final math's dependency chain can start earliest).
  - Final math on [128, K] fp32 columns: wide broadcast ops build C and
    M = C^T C; det(M - qI) comes from the characteristic-poly identity
    -2.5 q^3 + 0.5 q tr(M^2) + det(C)^2; cos(acos(r)/3 + phase) roots
    come from Newton on 4c^3 - 3c = r (cubic init, 2 iterations);
    Kabsch det sign, RMSD.
"""

import numpy as np

import concourse.bass as bass
import concourse.mybir as mybir
from concourse.tile import TileContext, ScopedClock

F32 = mybir.dt.float32
BF16 = mybir.dt.bfloat16
FP8 = mybir.dt.float8e4
OP_DT = FP8  # gram operand dtype (host-cast)
OP = mybir.AluOpType
AF = mybir.ActivationFunctionType

N_CORES = 8
ROWS = 128          # rows per tile == final partitions
GROUPS = 8          # row-groups per tile
R = 16              # rows per group
CW = 7              # cols per row: x0 x1 x2 y0 y1 y2 1
GW = R * CW         # group operand width = 112
CHUNK = 128         # atoms per matmul pass (contraction dim)
NSTAT = CW * CW     # 49 stats per row
K = 4               # classes (tiles per core)


# ---------------------------------------------------------------------------
# TileContext tail patch: this walrus build accepts at most ONE sync-wait
# command per instruction and no sem-eq waits, so the stock drain + EVSEM
# butterfly fails codegen. Emit a ge-wait-only tail instead.
# ---------------------------------------------------------------------------
def _patched_drain_and_barrier(self, tick_clock, wait_clock):
    nc = self.nc
    dummy = nc.gpsimd.nop()
    wait_clock.add_sem_waits(dummy.ins, ScopedClock({None: tick_clock.global_clock}))
    waits = list(dummy.ins.sync_info.on_wait) if dummy.ins.sync_info else []
    if dummy.ins.sync_info:
        dummy.ins.sync_info = mybir.SyncInfo(on_wait=[], on_update=[])

    bsem = nc.alloc_semaphore(f"tail_bsem_{nc.next_id()}")
    dsem = nc.alloc_semaphore(f"tail_dsem_{nc.next_id()}")
    n_eng = 0
    for eng in nc.engines.values():
        eng.drain()
        eng.sem_inc(bsem, 1)
        n_eng += 1
    nc.gpsimd.wait_ge(bsem, n_eng)
    for w in waits:
        n = nc.gpsimd.nop()
        n.ins.sync_info = mybir.SyncInfo(on_wait=[w], on_update=[])
    nc.gpsimd.sem_inc(dsem, 1)
    for eng in nc.engines.values():
        if eng is not nc.gpsimd:
            eng.wait_ge(dsem, 1)

    popped = nc._tile_sem_poison_stack.pop()
    assert popped is self._sem_poison
    nc.clear_and_free_semaphores(list(self.sems.allocated().values()))
    nc.gpsimd.sem_clear(bsem)
    nc.gpsimd.sem_clear(dsem)


def install_tile_patch():
    TileContext._drain_and_barrier = _patched_drain_and_barrier


# ---------------------------------------------------------------------------
# BIR post-pass: split multi-wait sync infos onto NoOps (walrus accepts at
# most one sync-wait command per instruction, none on Drain).
# ---------------------------------------------------------------------------
_orig_to_json_bytes = bass.Bass.to_json_bytes


def _split_multiwait_json(self) -> bytes:
    import json

    raw = _orig_to_json_bytes(self)
    m = json.loads(raw)
    ctr = 0
    changed = False
    for f in m.get("functions", []):
        for blk in f.get("blocks", []):
            insts = blk.get("instructions", [])
            out = []
            for inst in insts:
                si = inst.get("sync_info")
                ow = (si or {}).get("on_wait") or []
                opc = str(inst.get("opcode", inst.get("type", "")))
                limit = 0 if opc == "Drain" else 1
                if len(ow) > limit:
                    keep = ow[len(ow) - limit :] if limit else []
                    moved = ow[: len(ow) - limit] if limit else ow
                    for w in moved:
                        ctr += 1
                        out.append(
                            {
                                "debug": inst.get("debug", 0),
                                "engine": inst["engine"],
                                "ins": [],
                                "name": f"WS-{ctr}-{inst['name']}",
                                "opcode": "NoOp",
                                "outs": [],
                                "sync_info": {"on_update": [], "on_wait": [w]},
                            }
                        )
                    si["on_wait"] = keep
                    changed = True
                out.append(inst)
            blk["instructions"] = out
    if not changed:
        return raw
    return json.dumps(m).encode()


bass.Bass.to_json_bytes = _split_multiwait_json


# ---------------------------------------------------------------------------
# Final math emitter on [128, K] fp32 column tiles.
# final layout: [128 rows, (t: K)(kk: 7)(cc: 7)] fp32
#   G(kk, cc) = sum_n op[n, kk] op[n, cc] per row (kk,cc in 0..5 = comps,
#   6 = ones => sums). Columns for class t at offset t*49.
# ---------------------------------------------------------------------------
class _FM:
    def __init__(self, nc, pool, Kn, prefix=""):
        self.nc = nc
        self.pool = pool
        self.K = Kn
        self.n = 0
        self.prefix = prefix
        self._consts = {}

    def const_col(self, val):
        val = float(val)
        if val in self._consts:
            return self._consts[val]
        i = len(self._consts)
        t = self.pool.tile([ROWS, 1], F32, tag=f"fmc{i}", name=f"fmc{i}")
        self.nc.vector.memset(t[:], val)
        self._consts[val] = t[:]
        return t[:]

    def t(self, w=None):
        self.n += 1
        nm = f"fm{self.prefix}{self.n}"
        return self.pool.tile([ROWS, w or self.K], F32, tag=nm, name=nm)

    def tt(self, a, b, op):
        o = self.t()
        self.nc.vector.tensor_tensor(o[:], a, b, op)
        return o[:]

    def mul(self, a, b):
        return self.tt(a, b, OP.mult)

    def add(self, a, b):
        return self.tt(a, b, OP.add)

    def sub(self, a, b):
        return self.tt(a, b, OP.subtract)

    def ts(self, a, s, op):
        o = self.t()
        self.nc.vector.tensor_scalar(o[:], a, float(s), None, op)
        return o[:]

    def ts2(self, a, s1, s2, op0, op1):
        o = self.t()
        self.nc.vector.tensor_scalar(o[:], a, float(s1), float(s2), op0, op1)
        return o[:]

    def stt(self, a, s, b, op0, op1):
        """(a op0 s) op1 b"""
        o = self.t()
        self.nc.vector.scalar_tensor_tensor(o[:], a, float(s), b, op0, op1)
        return o[:]

    def act(self, a, func, bias=0.0, scale=1.0):
        o = self.t()
        if isinstance(bias, float) and bias not in (0.0, 1.0) and func != AF.Copy:
            bias = self.const_col(bias)
        self.nc.scalar.activation(o[:], a, func, bias=bias, scale=scale)
        return o[:]

    def recip(self, a):
        o = self.t()
        self.nc.vector.reciprocal(o[:], a)
        return o[:]


def _emit_math_pair(nc, fm, final_t, meta_ap, out_ap, Kn, t0, Kp):
    """Wide-op final math for classes [t0, t0+Kp)."""
    fv = final_t[:].rearrange("p (t k c) -> p t k c", t=Kn, k=CW)[
        :, t0 : t0 + Kp, :, :
    ]
    fvf = final_t[:].rearrange("p (t c) -> p t c", t=Kn)[
        :, t0 : t0 + Kp, :
    ]

    def W(w):  # fresh wide tile
        return fm.t(w)

    rn = fm.recip(meta_ap)  # [128, Kp]
    rn_b3 = rn[:, :, None].broadcast_to([ROWS, Kp, 3])

    P = fv[:, :, 0:3, 3:6]          # [128, Kp, 3, 3]
    Sall = fv[:, :, 6, 0:6]         # [128, Kp, 6]
    Sy = fv[:, :, 6, 3:6]
    rn_b6 = rn[:, :, None].broadcast_to([ROWS, Kp, 6])

    sn_t = W(Kp * 6)
    sn6 = sn_t[:].rearrange("p (t c) -> p t c", t=Kp)
    nc.vector.tensor_tensor(sn6, Sall, rn_b6, OP.mult)
    sxn = sn6[:, :, 0:3]

    t1_t = W(Kp * 9)
    t1 = t1_t[:].rearrange("p (t i j) -> p t i j", t=Kp, i=3)
    nc.vector.tensor_tensor(
        t1, sxn[:, :, :, None].broadcast_to([ROWS, Kp, 3, 3]),
        Sy[:, :, None, :].broadcast_to([ROWS, Kp, 3, 3]), OP.mult)
    C_t = W(Kp * 9)
    C = C_t[:].rearrange("p (t i j) -> p t i j", t=Kp, i=3)
    nc.vector.tensor_tensor(C, P, t1, OP.subtract)

    def Cij(i, j):
        return C[:, :, i, j]

    # M = C^T C via 3 outer products
    M_t = W(Kp * 9)
    M = M_t[:].rearrange("p (t a b) -> p t a b", t=Kp, a=3)
    tmp_t = W(Kp * 9)
    tmp = tmp_t[:].rearrange("p (t a b) -> p t a b", t=Kp, a=3)
    for i in range(3):
        Ci = C[:, :, i, :]
        dst = M if i == 0 else tmp
        nc.vector.tensor_tensor(
            dst, Ci[:, :, :, None].broadcast_to([ROWS, Kp, 3, 3]),
            Ci[:, :, None, :].broadcast_to([ROWS, Kp, 3, 3]), OP.mult)
        if i > 0:
            nc.vector.tensor_tensor(M, M, tmp, OP.add)

    Mf = M_t[:].rearrange("p (t ab) -> p t ab", t=Kp)
    Mdiag = Mf[:, :, 0:9:4]  # [128, 2, 3]

    # q = trM/3
    q = fm.add(Mdiag[:, :, 0], Mdiag[:, :, 1])
    q = fm.stt(Mdiag[:, :, 2], 1.0, q, OP.mult, OP.add)
    q = fm.ts(q, 1.0 / 3.0, OP.mult)

    # trM2 = sum M*M ; p2 = trM2 - 3 q^2
    MM_t = W(Kp * 9)
    nc.vector.tensor_tensor(MM_t[:], M_t[:], M_t[:], OP.mult)
    trM2 = fm.t()
    nc.vector.tensor_reduce(
        trM2[:], MM_t[:].rearrange("p (t ab) -> p t ab", t=Kp),
        mybir.AxisListType.X, OP.add)
    qq = fm.mul(q, q)
    p2 = fm.stt(qq, -3.0, trM2[:], OP.mult, OP.add)
    p2c = fm.ts2(p2, 1.0 / 6.0, 1e-30, OP.mult, OP.max)
    p = fm.act(p2c, AF.Sqrt)

    # --- detC, detC^2, sign (DVE; pool per-op overhead hurts the chain) ---
    def gtt(a, b, op):
        o = fm.t()
        nc.vector.tensor_tensor(o[:], a, b, op)
        return o[:]

    gm0 = gtt(Cij(1, 1), Cij(2, 2), OP.mult)
    gm0b = gtt(Cij(1, 2), Cij(2, 1), OP.mult)
    gm0 = gtt(gm0, gm0b, OP.subtract)
    gm1 = gtt(Cij(1, 0), Cij(2, 2), OP.mult)
    gm1b = gtt(Cij(1, 2), Cij(2, 0), OP.mult)
    gm1 = gtt(gm1, gm1b, OP.subtract)
    gm2 = gtt(Cij(1, 0), Cij(2, 1), OP.mult)
    gm2b = gtt(Cij(1, 1), Cij(2, 0), OP.mult)
    gm2 = gtt(gm2, gm2b, OP.subtract)
    d0 = gtt(Cij(0, 0), gm0, OP.mult)
    d1 = gtt(Cij(0, 1), gm1, OP.mult)
    d2 = gtt(Cij(0, 2), gm2, OP.mult)
    detC = gtt(gtt(d0, d1, OP.subtract), d2, OP.add)
    detC2 = gtt(detC, detC, OP.mult)
    dneg = fm.t()
    nc.vector.tensor_scalar(dneg[:], detC, 0.0, None, OP.is_lt)

    # detKq = det(M - qI) = -2.5 q^3 + 0.5 q trM2 + detC^2
    q3 = fm.mul(qq, q)
    a_ = fm.mul(q, trM2[:])
    t_ = fm.stt(a_, 0.5, detC2, OP.mult, OP.add)
    detKq = fm.stt(q3, -2.5, t_, OP.mult, OP.add)

    # r = 0.5 detKq / p^3 clamped
    rp = fm.recip(p)
    rp3 = fm.mul(fm.mul(rp, rp), rp)
    r = fm.stt(detKq, 0.5, rp3, OP.mult, OP.mult)
    r = fm.ts2(r, 1.0, -1.0, OP.min, OP.max)

    # Newton on 4c^3-3c=r for c1 (cos(phi)) and c3 (cos(phi+2pi/3)), packed
    # cubic init c1 = E(r^2) + r O(r^2); c3(r) = -c1(-r) = -E + r O
    E1, E0 = -0.07910172, 0.87011722
    O1, O0 = 0.06293734, 0.15509478
    rr = fm.mul(r, r)
    cpack_t = W(2 * Kp)
    cpack = cpack_t[:].rearrange("p (s t) -> p s t", s=2)
    Ev = fm.ts2(rr, E1, E0, OP.mult, OP.add)
    Ov = fm.ts2(rr, O1, O0, OP.mult, OP.add)
    rO = fm.mul(r, Ov)
    nc.vector.tensor_tensor(cpack[:, 0, :], Ev, rO, OP.add)
    nc.vector.tensor_tensor(cpack[:, 1, :], rO, Ev, OP.subtract)
    r_b = r[:, None, :].broadcast_to([ROWS, 2, Kp])
    for _ in range(2):
        c2 = fm.t(2 * Kp)
        nc.vector.tensor_tensor(c2[:], cpack_t[:], cpack_t[:], OP.mult)
        c3 = fm.t(2 * Kp)
        nc.vector.tensor_tensor(c3[:], c2[:], cpack_t[:], OP.mult)
        num = fm.t(2 * Kp)
        nc.vector.scalar_tensor_tensor(
            num[:].rearrange("p (s t) -> p s t", s=2),
            c3[:].rearrange("p (s t) -> p s t", s=2), 8.0, r_b,
            OP.mult, OP.add)
        den = fm.t(2 * Kp)
        nc.vector.tensor_scalar(den[:], c2[:], 12.0, -3.0, OP.mult, OP.add)
        rec = fm.t(2 * Kp)
        nc.vector.reciprocal(rec[:], den[:])
        nc.vector.tensor_tensor(cpack_t[:], num[:], rec[:], OP.mult)

    # lambdas: l1 = q + 2p c1 ; l3 = q + 2p c3 ; l2 = 3q - l1 - l3
    p2x = fm.ts(p, 2.0, OP.mult)
    lpack_t = W(3 * Kp)
    lpack = lpack_t[:].rearrange("p (s t) -> p s t", s=3)
    p2x_b = p2x[:, None, :].broadcast_to([ROWS, 2, Kp])
    q_b = q[:, None, :].broadcast_to([ROWS, 2, Kp])
    tl_t = W(2 * Kp)
    tl = tl_t[:].rearrange("p (s t) -> p s t", s=2)
    nc.vector.tensor_tensor(tl, p2x_b, cpack, OP.mult)
    nc.vector.tensor_tensor(lpack[:, 0:2, :], q_b, tl, OP.add)
    t_l2 = fm.stt(q, 3.0, lpack[:, 0, :], OP.mult, OP.subtract)
    nc.vector.tensor_tensor(lpack[:, 2, :], t_l2, lpack[:, 1, :], OP.subtract)
    lmax = fm.t(3 * Kp)
    nc.vector.tensor_scalar(lmax[:], lpack_t[:], 0.0, None, OP.max)
    spack_t = fm.t(3 * Kp)
    nc.scalar.activation(spack_t[:], lmax[:], AF.Sqrt)
    spack = spack_t[:].rearrange("p (s t) -> p s t", s=3)

    # gx + gy: one reduce over all six diag cols; packed sum-sq reduce
    Qsum = fm.t()
    nc.vector.tensor_reduce(Qsum[:], fvf[:, :, 0:41:8], mybir.AxisListType.X, OP.add)
    snS_t = W(Kp * 6)
    nc.vector.tensor_tensor(
        snS_t[:].rearrange("p (t c) -> p t c", t=Kp), sn6, Sall, OP.mult)
    s2sum = fm.t()
    nc.vector.tensor_reduce(
        s2sum[:], snS_t[:].rearrange("p (t c) -> p t c", t=Kp),
        mybir.AxisListType.X, OP.add)
    g = fm.sub(Qsum[:], s2sum[:])
    tr = fm.add(fm.add(spack[:, 0, :], spack[:, 2, :]), spack[:, 1, :])
    tr = fm.stt(fm.mul(dneg[:], spack[:, 1, :]), -2.0, tr, OP.mult, OP.add)


    diff = fm.stt(tr, -2.0, g, OP.mult, OP.add)
    msd = fm.mul(diff, rn)
    nc.scalar.activation(out_ap, fm.ts(msd, 0.0, OP.max), AF.Sqrt)


# ---------------------------------------------------------------------------
# Program builder
# ---------------------------------------------------------------------------
def build_program(chunks, cfg=None):
    """chunks: per-class chunk counts (len K). Returns nc."""
    cfg = cfg or {}
    do_mm = cfg.get("mm", True)
    do_extract = cfg.get("extract", True)
    do_math = cfg.get("math", True)
    Kn = len(chunks)
    install_tile_patch()
    nc = bass.Bass()
    op_dt = FP8 if cfg.get("fp8", True) else BF16
    op_d = [
        nc.dram_tensor(f"op{t}", [ROWS, chunks[t] * GROUPS * GW], op_dt,
                       kind="ExternalInput")
        for t in range(Kn)
    ]
    sel_d = nc.dram_tensor("sel", [GW, R * CW], BF16, kind="ExternalInput")
    meta_d = nc.dram_tensor("meta", [ROWS, Kn], F32, kind="ExternalInput")
    out_d = nc.dram_tensor("out", [ROWS, Kn], F32, kind="ExternalOutput")

    with TileContext(nc) as tc:
        with (
            tc.tile_pool(name="const", bufs=1) as constp,
            tc.tile_pool(name="ops", bufs=1) as opp,
            tc.tile_pool(name="gsb", bufs=2) as gsbp,
            tc.tile_pool(name="ext", bufs=1) as extp,
            tc.tile_pool(name="fmp", bufs=1) as fmp,
            tc.tile_pool(name="psA", bufs=2, space="PSUM") as psA,
            tc.tile_pool(name="psB", bufs=2, space="PSUM") as psB,
        ):
            sel_t = constp.tile([GW, R * CW], BF16)
            nc.sync.dma_start(out=sel_t[:], in_=sel_d[:])
            meta_t = constp.tile([ROWS, Kn], F32)
            nc.sync.dma_start(out=meta_t[:], in_=meta_d[:])

            # staging for rows: ext [7, (r 16)(g 8)(t 2)(c 7)] per pair
            exts = [
                extp.tile([CW, R * GROUPS * 2 * CW], F32, name=f"extp{p}")
                for p in range(Kn // 2)
            ]
            final_t = fmp.tile([ROWS, Kn * NSTAT], F32)

            # PE pstate warmup: keep PE busy during the first load so the
            # ramp to full clock completes before the first gram matmul.
            nwarm = cfg.get("warmup", 100)
            if nwarm:
                wv = psB.tile([128, 1024], F32, tag="ps2")
                for i in range(nwarm):
                    nc.tensor.matmul(
                        wv[0:CW, 0:CW], sel_t[:, 0:CW], sel_t[:, 0:CW],
                        start=True, stop=True, skip_group_check=True,
                    )

            op_t = []
            for t in range(Kn):
                op = opp.tile([ROWS, chunks[t] * GROUPS * GW], op_dt, name=f"op{t}")
                half = (GROUPS // 2) * chunks[t] * GW
                nc.sync.dma_start(out=op[:, 0:half], in_=op_d[t][:, 0:half])
                nc.sync.dma_start(out=op[:, half:], in_=op_d[t][:, half:])
                op_t.append(op)

            out_t = fmp.tile([ROWS, Kn], F32)
            grams = {}
            gsbs = {}
            evs = {}
            fvv = final_t[:].rearrange("p (t k c) -> p t k c", t=Kn, k=CW)

            def emit_grams(t):
                Ct = chunks[t]
                op = op_t[t]
                gram = psA.tile([128, 1024], F32, tag="gram")
                gv = gram[:].rearrange("p (g w) -> p g w", g=GROUPS)
                use_dr = cfg.get("double_row", True) and op_dt == FP8
                for g in range(GROUPS):
                    if use_dr:
                        npair = Ct // 2
                        for c in range(npair):
                            sl = op[
                                :, (g * Ct + 2 * c) * GW : (g * Ct + 2 * c + 2) * GW
                            ].rearrange("p (k w) -> p k w", k=2)
                            nc.tensor.matmul(
                                gv[0:GW, g, 0:GW], sl, sl,
                                start=(c == 0), stop=(c == npair - 1 and Ct % 2 == 0),
                                skip_group_check=True,
                                perf_mode=mybir.MatmulPerfMode.DoubleRow,
                            )
                        if Ct % 2:
                            sl = op[:, (g * Ct + Ct - 1) * GW : (g * Ct + Ct) * GW]
                            nc.tensor.matmul(
                                gv[0:GW, g, 0:GW], sl, sl,
                                start=(Ct == 1), stop=True,
                                skip_group_check=True,
                            )
                    else:
                        for c in range(Ct):
                            sl = op[:, (g * Ct + c) * GW : (g * Ct + c + 1) * GW]
                            nc.tensor.matmul(
                                gv[0:GW, g, 0:GW], sl, sl,
                                start=(c == 0), stop=(c == Ct - 1),
                                skip_group_check=True,
                            )
                grams[t] = gv
                # Act copy1 queued immediately (runs when grams stop)
                gram_sb = gsbp.tile([GW, GROUPS * GW], BF16, tag="gramsb")
                gsv = gram_sb[:].rearrange("p (g w) -> p g w", g=GROUPS)
                nc.scalar.activation(gsv[:, :, :], gv[0:GW, :, 0:GW], AF.Copy)
                gsbs[t] = gsv

            def emit_selects(t):
                gsv = gsbs[t]
                ps2 = psB.tile([128, 1024], F32, tag="ps2")
                p2v = ps2[:].rearrange("p (r w) -> p r w", r=R)
                for r in range(R):
                    rhs = gsv[:, :, CW * r : CW * r + CW]
                    lhsT = sel_t[:, CW * r : CW * r + CW]
                    nc.tensor.matmul(
                        p2v[0:CW, r, 0 : GROUPS * CW], lhsT, rhs,
                        start=True, stop=True, skip_group_check=True,
                    )
                pair, tp = divmod(t, 2)
                ev = exts[pair][:].rearrange(
                    "p (r g t c) -> p r g t c", r=R, g=GROUPS, t=2
                )
                nc.scalar.activation(
                    ev[:, :, :, tp, :],
                    p2v[0:CW, :, 0 : GROUPS * CW].rearrange(
                        "p r (g c) -> p r g c", g=GROUPS
                    ),
                    AF.Copy,
                )
                evs[pair] = ev

            def emit_finals(t, both=False):
                pair, tp = divmod(t, 2)
                ev = evs[pair]
                for kkc in [6, 0, 1, 2, 3, 4, 5]:
                    eng = nc.gpsimd if kkc in (2, 5) else nc.sync
                    if both:
                        eng.dma_start(
                            out=fvv[:, 2 * pair : 2 * pair + 2, kkc, :],
                            in_=ev[kkc : kkc + 1, :, :, :, :],
                        )
                    else:
                        eng.dma_start(
                            out=fvv[:, t : t + 1, kkc, :],
                            in_=ev[kkc : kkc + 1, :, :, tp : tp + 1, :],
                        )

            if do_mm and do_extract:
                emit_grams(0)
                emit_grams(1)
                emit_selects(0)
                emit_grams(2)
                emit_selects(1)
                emit_finals(1, both=True)
                emit_grams(3)
                emit_selects(2)
                emit_selects(3)
                emit_finals(3, both=True)
                if do_math:
                    fm = _FM(nc, fmp, Kn, prefix="m_")
                    _emit_math_pair(
                        nc, fm, final_t, meta_t[:], out_t[:], Kn, 0, Kn
                    )
            elif do_mm:
                for t in range(Kn):
                    emit_grams(t)
            if not (do_mm and do_extract and do_math):
                nc.vector.memset(out_t[:], 0.0)
            nc.sync.dma_start(out=out_d[:], in_=out_t[:])
    return nc


# ---------------------------------------------------------------------------
# Host side
# ---------------------------------------------------------------------------
def plan_shards(num_atoms, n_classes=K):
    """Sort rows into 32 global tiles of 128; snake-assign 4 tiles per core.

    Returns (order, assign, core_chunks): assign[c] = 4 global tile indices
    (processed big-first), core_chunks[c] = matching chunk counts.
    """
    B = num_atoms.shape[0]
    ntiles = B // ROWS
    assert ntiles == N_CORES * n_classes
    order = np.argsort(num_atoms, kind="stable")
    nas = num_atoms[order]
    tile_chunks = [
        int((int(nas[(i + 1) * ROWS - 1]) + CHUNK - 1) // CHUNK)
        for i in range(ntiles)
    ]
    assign = []
    core_chunks = []
    for c in range(N_CORES):
        tiles = [c, 15 - c, 16 + c, 31 - c]
        tiles.sort(key=lambda t: -tile_chunks[t])  # big-first
        assign.append(tiles)
        core_chunks.append([tile_chunks[t] for t in tiles])
    return order, assign, core_chunks


def _pack_tile(x, y, na, Ct):
    """x, y: [128, nmax, 3] f32 (row-major positions), na: [128] int.
    Returns op [128, Ct, GROUPS, GW] f32 with atoms on dim 0 (partitions)."""
    nmax = x.shape[1]
    cap = Ct * CHUNK
    # data [b, n, 7]
    d = np.zeros((ROWS, cap, CW), np.float32)
    ncl = min(cap, nmax)
    d[:, :ncl, 0:3] = x[:, :ncl, :]
    d[:, :ncl, 3:6] = y[:, :ncl, :]
    mask = (np.arange(cap)[None, :] < na[:, None]).astype(np.float32)
    d[:, :, 0:6] *= mask[:, :, None]
    d[:, :, 6] = 1.0
    # op[p, g, c, 7r+k] = d[8r+g, c*128+p, k]   (group-major for strip loads)
    d = d.reshape(ROWS, Ct, CHUNK, CW)            # [b, c, p, k]
    d = d.transpose(2, 1, 0, 3)                   # [p, c, b, k]
    d = d.reshape(CHUNK, Ct, R, GROUPS, CW)       # [p, c, r, g, k]  (b = 8r+g)
    d = d.transpose(0, 3, 1, 2, 4)                # [p, g, c, r, k]
    return np.ascontiguousarray(d.reshape(CHUNK, GROUPS, Ct, GW))


def _op_np_dtype():
    return mybir.dt.np(OP_DT)


def shard_inputs(coords_input, coords_target, num_atoms, order, assign, core_chunks):
    import ml_dtypes

    B, ncols = coords_input.shape
    nmax = ncols // 3
    sel = np.zeros((GW, R * CW), np.float32)
    for j in range(R * CW):
        sel[j, j] = 1.0
    sel = sel.astype(ml_dtypes.bfloat16)
    in_maps = []
    core_row_idx = []
    for c in range(N_CORES):
        m = {"sel": sel}
        idx_all = []
        Kn = len(assign[c])
        meta = np.zeros((ROWS, Kn), np.float32)
        for t in range(Kn):
            gt = assign[c][t]
            idx = order[gt * ROWS : (gt + 1) * ROWS]
            idx_all.append(idx)
            na = num_atoms[idx]
            meta[:, t] = na.astype(np.float32)
            x = coords_input[idx].reshape(ROWS, nmax, 3)
            y = coords_target[idx].reshape(ROWS, nmax, 3)
            op = _pack_tile(x, y, na, core_chunks[c][t])
            m[f"op{t}"] = np.ascontiguousarray(
                op.reshape(CHUNK, -1)
            ).astype(_op_np_dtype())
        m["meta"] = meta
        in_maps.append(m)
        core_row_idx.append(np.concatenate(idx_all))
    return in_maps, core_row_idx


def unshard_outputs(results, core_row_idx, B):
    out = np.empty(B, dtype=np.float32)
    for c in range(N_CORES):
        o = results[c]["out"]  # [ROWS, K]
        out[core_row_idx[c]] = o.T.reshape(-1)
    return out


# ---------------------------------------------------------------------------
# Entry point
# ---------------------------------------------------------------------------
_PROG_CACHE = {}


def _get_program(chunks):
    key = tuple(chunks)
    if key not in _PROG_CACHE:
        _PROG_CACHE[key] = build_program(list(chunks))
    return _PROG_CACHE[key]


def kernel(coords_input, coords_target, num_atoms):
    from concourse.bass_utils import run_bass_kernel_spmd

    x = np.ascontiguousarray(np.asarray(coords_input, dtype=np.float32))
    y = np.ascontiguousarray(np.asarray(coords_target, dtype=np.float32))
    na = np.asarray(num_atoms).astype(np.int64)
    B, ncols = x.shape
    Kn = B // (N_CORES * ROWS)
    assert B == N_CORES * ROWS * Kn, f"unsupported batch {B}"

    order, assign, core_chunks = plan_shards(na, n_classes=Kn)
    in_maps, core_row_idx = shard_inputs(x, y, na, order, assign, core_chunks)
    # group cores by identical chunk tuples -> one program per group
    groups = {}
    for c in range(N_CORES):
        groups.setdefault(tuple(core_chunks[c]), []).append(c)
    results = [None] * N_CORES
    for chunks, cores in groups.items():
        nc = _get_program(chunks)
        res = run_bass_kernel_spmd(
            nc, [in_maps[c] for c in cores], core_ids=list(range(len(cores)))
        )
        for i, c in enumerate(cores):
            results[c] = res.results[i]
    out = unshard_outputs(results, core_row_idx, B)
    return out.astype(np.float32)


# revision 34
# speedup vs baseline: 1.0064x; 1.0064x over previous
"""Bass/Trainium2 kernel for batched masked-Kabsch RMSD (nn_Coords2RMSD).

PE-centric design, one program per distinct per-core shape (8 cores):
  - Host sorts rows by num_atoms into 32 tiles of 128 rows and
    snake-assigns 4 tiles per core (big-first). Per tile, coords are
    repacked TRANSPOSED into fp8e4m3: atoms on SBUF partitions; for each
    group of 16 rows a 112-column operand [x y z X Y Z 1] per row.
    Padding atoms are zeroed on the host; the ones column makes the Gram
    matrix carry the masked sums.
  - Per (group, pair-of-128-atom-chunks) ONE symmetric DoubleRow fp8
    matmul op^T @ op accumulates in PSUM: the diagonal 7x7 blocks hold
    all 21 per-row statistics (cross-covariance, |x|^2, |y|^2, sums).
    A warmup matmul burst pins the PE pstate ramp before the real work.
  - Extraction: Act copies PSUM->SBUF (bf16), 16 identity-select
    matmuls gather the diagonal slot blocks into a second PSUM, Act
    copies them to a staging buffer, and 7 strided DMAs per tile-pair
    transpose [slot-comp, row] -> [row, stats] (sums row first so the
    final math's dependency chain can start earliest).
  - Final math on [128, K] fp32 columns: wide broadcast ops build C and
    M = C^T C; det(M - qI) comes from the characteristic-poly identity
    -2.5 q^3 + 0.5 q tr(M^2) + det(C)^2; cos(acos(r)/3 + phase) roots
    come from Newton on 4c^3 - 3c = r (cubic init, 2 iterations);
    Kabsch det sign, RMSD.
"""

import numpy as np

import concourse.bass as bass
import concourse.mybir as mybir
from concourse.tile import TileContext, ScopedClock

F32 = mybir.dt.float32
BF16 = mybir.dt.bfloat16
FP8 = mybir.dt.float8e4
OP_DT = FP8  # gram operand dtype (host-cast)
OP = mybir.AluOpType
AF = mybir.ActivationFunctionType

N_CORES = 8
ROWS = 128          # rows per tile == final partitions
GROUPS = 8          # row-groups per tile
R = 16              # rows per group
CW = 7              # cols per row: x0 x1 x2 y0 y1 y2 1
GW = R * CW         # group operand width = 112
CHUNK = 128         # atoms per matmul pass (contraction dim)
NSTAT = CW * CW     # 49 stats per row
K = 4               # classes (tiles per core)


# ---------------------------------------------------------------------------
# TileContext tail patch: this walrus build accepts at most ONE sync-wait
# command per instruction and no sem-eq waits, so the stock drain + EVSEM
# butterfly fails codegen. Emit a ge-wait-only tail instead.
# ---------------------------------------------------------------------------
def _patched_drain_and_barrier(self, tick_clock, wait_clock):
    nc = self.nc
    dummy = nc.gpsimd.nop()
    wait_clock.add_sem_waits(dummy.ins, ScopedClock({None: tick_clock.global_clock}))
    waits = list(dummy.ins.sync_info.on_wait) if dummy.ins.sync_info else []
    if dummy.ins.sync_info:
        dummy.ins.sync_info = mybir.SyncInfo(on_wait=[], on_update=[])

    bsem = nc.alloc_semaphore(f"tail_bsem_{nc.next_id()}")
    dsem = nc.alloc_semaphore(f"tail_dsem_{nc.next_id()}")
    engs = list(nc.engines.values())
    n_eng = 0
    for i, eng in enumerate(engs):
        for w in waits[i::len(engs)]:
            n = eng.nop()
            n.ins.sync_info = mybir.SyncInfo(on_wait=[w], on_update=[])
        eng.drain()
        eng.sem_inc(bsem, 1)
        n_eng += 1
    nc.gpsimd.wait_ge(bsem, n_eng)
    nc.gpsimd.sem_inc(dsem, 1)
    for eng in nc.engines.values():
        if eng is not nc.gpsimd:
            eng.wait_ge(dsem, 1)

    popped = nc._tile_sem_poison_stack.pop()
    assert popped is self._sem_poison
    nc.clear_and_free_semaphores(list(self.sems.allocated().values()))
    nc.gpsimd.sem_clear(bsem)
    nc.gpsimd.sem_clear(dsem)


def install_tile_patch():
    TileContext._drain_and_barrier = _patched_drain_and_barrier


# ---------------------------------------------------------------------------
# BIR post-pass: split multi-wait sync infos onto NoOps (walrus accepts at
# most one sync-wait command per instruction, none on Drain).
# ---------------------------------------------------------------------------
_orig_to_json_bytes = bass.Bass.to_json_bytes


def _split_multiwait_json(self) -> bytes:
    import json

    raw = _orig_to_json_bytes(self)
    m = json.loads(raw)
    ctr = 0
    changed = False
    for f in m.get("functions", []):
        for blk in f.get("blocks", []):
            insts = blk.get("instructions", [])
            out = []
            for inst in insts:
                si = inst.get("sync_info")
                ow = (si or {}).get("on_wait") or []
                opc = str(inst.get("opcode", inst.get("type", "")))
                limit = 0 if opc == "Drain" else 1
                if len(ow) > limit:
                    keep = ow[len(ow) - limit :] if limit else []
                    moved = ow[: len(ow) - limit] if limit else ow
                    for w in moved:
                        ctr += 1
                        out.append(
                            {
                                "debug": inst.get("debug", 0),
                                "engine": inst["engine"],
                                "ins": [],
                                "name": f"WS-{ctr}-{inst['name']}",
                                "opcode": "NoOp",
                                "outs": [],
                                "sync_info": {"on_update": [], "on_wait": [w]},
                            }
                        )
                    si["on_wait"] = keep
                    changed = True
                out.append(inst)
            blk["instructions"] = out
    if not changed:
        return raw
    return json.dumps(m).encode()


bass.Bass.to_json_bytes = _split_multiwait_json


# ---------------------------------------------------------------------------
# Final math emitter on [128, K] fp32 column tiles.
# final layout: [128 rows, (t: K)(kk: 7)(cc: 7)] fp32
#   G(kk, cc) = sum_n op[n, kk] op[n, cc] per row (kk,cc in 0..5 = comps,
#   6 = ones => sums). Columns for class t at offset t*49.
# ---------------------------------------------------------------------------
class _FM:
    def __init__(self, nc, pool, Kn, prefix=""):
        self.nc = nc
        self.pool = pool
        self.K = Kn
        self.n = 0
        self.prefix = prefix
        self._consts = {}

    def const_col(self, val):
        val = float(val)
        if val in self._consts:
            return self._consts[val]
        i = len(self._consts)
        t = self.pool.tile([ROWS, 1], F32, tag=f"fmc{i}", name=f"fmc{i}")
        self.nc.vector.memset(t[:], val)
        self._consts[val] = t[:]
        return t[:]

    def t(self, w=None):
        self.n += 1
        nm = f"fm{self.prefix}{self.n}"
        return self.pool.tile([ROWS, w or self.K], F32, tag=nm, name=nm)

    def tt(self, a, b, op):
        o = self.t()
        self.nc.vector.tensor_tensor(o[:], a, b, op)
        return o[:]

    def mul(self, a, b):
        return self.tt(a, b, OP.mult)

    def add(self, a, b):
        return self.tt(a, b, OP.add)

    def sub(self, a, b):
        return self.tt(a, b, OP.subtract)

    def ts(self, a, s, op):
        o = self.t()
        self.nc.vector.tensor_scalar(o[:], a, float(s), None, op)
        return o[:]

    def ts2(self, a, s1, s2, op0, op1):
        o = self.t()
        self.nc.vector.tensor_scalar(o[:], a, float(s1), float(s2), op0, op1)
        return o[:]

    def stt(self, a, s, b, op0, op1):
        """(a op0 s) op1 b"""
        o = self.t()
        self.nc.vector.scalar_tensor_tensor(o[:], a, float(s), b, op0, op1)
        return o[:]

    def act(self, a, func, bias=0.0, scale=1.0):
        o = self.t()
        if isinstance(bias, float) and bias not in (0.0, 1.0) and func != AF.Copy:
            bias = self.const_col(bias)
        self.nc.scalar.activation(o[:], a, func, bias=bias, scale=scale)
        return o[:]

    def recip(self, a):
        o = self.t()
        self.nc.vector.reciprocal(o[:], a)
        return o[:]


def _emit_math_pair(nc, fm, final_t, meta_ap, out_ap, Kn, t0, Kp):
    """Wide-op final math for classes [t0, t0+Kp)."""
    fv = final_t[:].rearrange("p (t k c) -> p t k c", t=Kn, k=CW)[
        :, t0 : t0 + Kp, :, :
    ]
    fvf = final_t[:].rearrange("p (t c) -> p t c", t=Kn)[
        :, t0 : t0 + Kp, :
    ]

    def W(w):  # fresh wide tile
        return fm.t(w)

    rn = fm.recip(meta_ap)  # [128, Kp]
    rn_b3 = rn[:, :, None].broadcast_to([ROWS, Kp, 3])

    P = fv[:, :, 0:3, 3:6]          # [128, Kp, 3, 3]
    Sall = fv[:, :, 6, 0:6]         # [128, Kp, 6]
    Sy = fv[:, :, 6, 3:6]
    rn_b6 = rn[:, :, None].broadcast_to([ROWS, Kp, 6])

    sn_t = W(Kp * 6)
    sn6 = sn_t[:].rearrange("p (t c) -> p t c", t=Kp)
    nc.vector.tensor_tensor(sn6, Sall, rn_b6, OP.mult)
    sxn = sn6[:, :, 0:3]

    t1_t = W(Kp * 9)
    t1 = t1_t[:].rearrange("p (t i j) -> p t i j", t=Kp, i=3)
    nc.vector.tensor_tensor(
        t1, sxn[:, :, :, None].broadcast_to([ROWS, Kp, 3, 3]),
        Sy[:, :, None, :].broadcast_to([ROWS, Kp, 3, 3]), OP.mult)
    C_t = W(Kp * 9)
    C = C_t[:].rearrange("p (t i j) -> p t i j", t=Kp, i=3)
    nc.vector.tensor_tensor(C, P, t1, OP.subtract)

    def Cij(i, j):
        return C[:, :, i, j]

    # M = C^T C via 3 outer products
    M_t = W(Kp * 9)
    M = M_t[:].rearrange("p (t a b) -> p t a b", t=Kp, a=3)
    tmp_t = W(Kp * 9)
    tmp = tmp_t[:].rearrange("p (t a b) -> p t a b", t=Kp, a=3)
    for i in range(3):
        Ci = C[:, :, i, :]
        dst = M if i == 0 else tmp
        nc.vector.tensor_tensor(
            dst, Ci[:, :, :, None].broadcast_to([ROWS, Kp, 3, 3]),
            Ci[:, :, None, :].broadcast_to([ROWS, Kp, 3, 3]), OP.mult)
        if i > 0:
            nc.vector.tensor_tensor(M, M, tmp, OP.add)

    Mf = M_t[:].rearrange("p (t ab) -> p t ab", t=Kp)
    Mdiag = Mf[:, :, 0:9:4]  # [128, 2, 3]

    # q = trM/3
    q = fm.add(Mdiag[:, :, 0], Mdiag[:, :, 1])
    q = fm.stt(Mdiag[:, :, 2], 1.0, q, OP.mult, OP.add)
    q = fm.ts(q, 1.0 / 3.0, OP.mult)

    # trM2 = sum M*M ; p2 = trM2 - 3 q^2
    MM_t = W(Kp * 9)
    nc.vector.tensor_tensor(MM_t[:], M_t[:], M_t[:], OP.mult)
    trM2 = fm.t()
    nc.vector.tensor_reduce(
        trM2[:], MM_t[:].rearrange("p (t ab) -> p t ab", t=Kp),
        mybir.AxisListType.X, OP.add)
    qq = fm.mul(q, q)
    p2 = fm.stt(qq, -3.0, trM2[:], OP.mult, OP.add)
    p2c = fm.ts2(p2, 1.0 / 6.0, 1e-30, OP.mult, OP.max)
    p = fm.act(p2c, AF.Sqrt)

    # --- detC, detC^2, sign (DVE; pool per-op overhead hurts the chain) ---
    def gtt(a, b, op):
        o = fm.t()
        nc.vector.tensor_tensor(o[:], a, b, op)
        return o[:]

    gm0 = gtt(Cij(1, 1), Cij(2, 2), OP.mult)
    gm0b = gtt(Cij(1, 2), Cij(2, 1), OP.mult)
    gm0 = gtt(gm0, gm0b, OP.subtract)
    gm1 = gtt(Cij(1, 0), Cij(2, 2), OP.mult)
    gm1b = gtt(Cij(1, 2), Cij(2, 0), OP.mult)
    gm1 = gtt(gm1, gm1b, OP.subtract)
    gm2 = gtt(Cij(1, 0), Cij(2, 1), OP.mult)
    gm2b = gtt(Cij(1, 1), Cij(2, 0), OP.mult)
    gm2 = gtt(gm2, gm2b, OP.subtract)
    d0 = gtt(Cij(0, 0), gm0, OP.mult)
    d1 = gtt(Cij(0, 1), gm1, OP.mult)
    d2 = gtt(Cij(0, 2), gm2, OP.mult)
    detC = gtt(gtt(d0, d1, OP.subtract), d2, OP.add)
    detC2 = gtt(detC, detC, OP.mult)
    dneg = fm.t()
    nc.vector.tensor_scalar(dneg[:], detC, 0.0, None, OP.is_lt)

    # detKq = det(M - qI) = -2.5 q^3 + 0.5 q trM2 + detC^2
    q3 = fm.mul(qq, q)
    a_ = fm.mul(q, trM2[:])
    t_ = fm.stt(a_, 0.5, detC2, OP.mult, OP.add)
    detKq = fm.stt(q3, -2.5, t_, OP.mult, OP.add)

    # r = 0.5 detKq / p^3 clamped
    rp = fm.recip(p)
    rp3 = fm.mul(fm.mul(rp, rp), rp)
    r = fm.stt(detKq, 0.5, rp3, OP.mult, OP.mult)
    r = fm.ts2(r, 1.0, -1.0, OP.min, OP.max)

    # Newton on 4c^3-3c=r for c1 (cos(phi)) and c3 (cos(phi+2pi/3)), packed
    # cubic init c1 = E(r^2) + r O(r^2); c3(r) = -c1(-r) = -E + r O
    E1, E0 = -0.07910172, 0.87011722
    O1, O0 = 0.06293734, 0.15509478
    rr = fm.mul(r, r)
    cpack_t = W(2 * Kp)
    cpack = cpack_t[:].rearrange("p (s t) -> p s t", s=2)
    Ev = fm.ts2(rr, E1, E0, OP.mult, OP.add)
    Ov = fm.ts2(rr, O1, O0, OP.mult, OP.add)
    rO = fm.mul(r, Ov)
    nc.vector.tensor_tensor(cpack[:, 0, :], Ev, rO, OP.add)
    nc.vector.tensor_tensor(cpack[:, 1, :], rO, Ev, OP.subtract)
    r_b = r[:, None, :].broadcast_to([ROWS, 2, Kp])
    for _ in range(2):
        c2 = fm.t(2 * Kp)
        nc.vector.tensor_tensor(c2[:], cpack_t[:], cpack_t[:], OP.mult)
        c3 = fm.t(2 * Kp)
        nc.vector.tensor_tensor(c3[:], c2[:], cpack_t[:], OP.mult)
        num = fm.t(2 * Kp)
        nc.vector.scalar_tensor_tensor(
            num[:].rearrange("p (s t) -> p s t", s=2),
            c3[:].rearrange("p (s t) -> p s t", s=2), 8.0, r_b,
            OP.mult, OP.add)
        den = fm.t(2 * Kp)
        nc.vector.tensor_scalar(den[:], c2[:], 12.0, -3.0, OP.mult, OP.add)
        rec = fm.t(2 * Kp)
        nc.vector.reciprocal(rec[:], den[:])
        nc.vector.tensor_tensor(cpack_t[:], num[:], rec[:], OP.mult)

    # lambdas: l1 = q + 2p c1 ; l3 = q + 2p c3 ; l2 = 3q - l1 - l3
    p2x = fm.ts(p, 2.0, OP.mult)
    lpack_t = W(3 * Kp)
    lpack = lpack_t[:].rearrange("p (s t) -> p s t", s=3)
    p2x_b = p2x[:, None, :].broadcast_to([ROWS, 2, Kp])
    q_b = q[:, None, :].broadcast_to([ROWS, 2, Kp])
    tl_t = W(2 * Kp)
    tl = tl_t[:].rearrange("p (s t) -> p s t", s=2)
    nc.vector.tensor_tensor(tl, p2x_b, cpack, OP.mult)
    nc.vector.tensor_tensor(lpack[:, 0:2, :], q_b, tl, OP.add)
    t_l2 = fm.stt(q, 3.0, lpack[:, 0, :], OP.mult, OP.subtract)
    nc.vector.tensor_tensor(lpack[:, 2, :], t_l2, lpack[:, 1, :], OP.subtract)
    lmax = fm.t(3 * Kp)
    nc.vector.tensor_scalar(lmax[:], lpack_t[:], 0.0, None, OP.max)
    spack_t = fm.t(3 * Kp)
    nc.scalar.activation(spack_t[:], lmax[:], AF.Sqrt)
    spack = spack_t[:].rearrange("p (s t) -> p s t", s=3)

    # gx + gy: one reduce over all six diag cols; packed sum-sq reduce
    Qsum = fm.t()
    nc.vector.tensor_reduce(Qsum[:], fvf[:, :, 0:41:8], mybir.AxisListType.X, OP.add)
    snS_t = W(Kp * 6)
    nc.vector.tensor_tensor(
        snS_t[:].rearrange("p (t c) -> p t c", t=Kp), sn6, Sall, OP.mult)
    s2sum = fm.t()
    nc.vector.tensor_reduce(
        s2sum[:], snS_t[:].rearrange("p (t c) -> p t c", t=Kp),
        mybir.AxisListType.X, OP.add)
    g = fm.sub(Qsum[:], s2sum[:])
    tr = fm.add(fm.add(spack[:, 0, :], spack[:, 2, :]), spack[:, 1, :])
    tr = fm.stt(fm.mul(dneg[:], spack[:, 1, :]), -2.0, tr, OP.mult, OP.add)


    diff = fm.stt(tr, -2.0, g, OP.mult, OP.add)
    msd = fm.mul(diff, rn)
    nc.scalar.activation(out_ap, fm.ts(msd, 0.0, OP.max), AF.Sqrt)


# ---------------------------------------------------------------------------
# Program builder
# ---------------------------------------------------------------------------
def build_program(chunks, cfg=None):
    """chunks: per-class chunk counts (len K). Returns nc."""
    cfg = cfg or {}
    do_mm = cfg.get("mm", True)
    do_extract = cfg.get("extract", True)
    do_math = cfg.get("math", True)
    Kn = len(chunks)
    install_tile_patch()
    nc = bass.Bass()
    op_dt = FP8 if cfg.get("fp8", True) else BF16
    op_d = [
        nc.dram_tensor(f"op{t}", [ROWS, chunks[t] * GROUPS * GW], op_dt,
                       kind="ExternalInput")
        for t in range(Kn)
    ]
    sel_d = nc.dram_tensor("sel", [GW, R * CW], BF16, kind="ExternalInput")
    meta_d = nc.dram_tensor("meta", [ROWS, Kn], F32, kind="ExternalInput")
    out_d = nc.dram_tensor("out", [ROWS, Kn], F32, kind="ExternalOutput")

    with TileContext(nc) as tc:
        with (
            tc.tile_pool(name="const", bufs=1) as constp,
            tc.tile_pool(name="ops", bufs=1) as opp,
            tc.tile_pool(name="gsb", bufs=2) as gsbp,
            tc.tile_pool(name="ext", bufs=1) as extp,
            tc.tile_pool(name="fmp", bufs=1) as fmp,
            tc.tile_pool(name="psA", bufs=2, space="PSUM") as psA,
            tc.tile_pool(name="psB", bufs=2, space="PSUM") as psB,
        ):
            sel_t = constp.tile([GW, R * CW], BF16)
            nc.sync.dma_start(out=sel_t[:], in_=sel_d[:])
            meta_t = constp.tile([ROWS, Kn], F32)
            nc.sync.dma_start(out=meta_t[:], in_=meta_d[:])

            # staging for rows: ext [7, (r 16)(g 8)(t 2)(c 7)] per pair
            exts = [
                extp.tile([CW, R * GROUPS * 2 * CW], F32, name=f"extp{p}")
                for p in range(Kn // 2)
            ]
            final_t = fmp.tile([ROWS, Kn * NSTAT], F32)

            # PE pstate warmup: keep PE busy during the first load so the
            # ramp to full clock completes before the first gram matmul.
            nwarm = cfg.get("warmup", 100)
            if nwarm:
                wv = psB.tile([128, 1024], F32, tag="ps2")
                for i in range(nwarm):
                    nc.tensor.matmul(
                        wv[0:CW, 0:CW], sel_t[:, 0:CW], sel_t[:, 0:CW],
                        start=True, stop=True, skip_group_check=True,
                    )

            op_t = []
            for t in range(Kn):
                op = opp.tile([ROWS, chunks[t] * GROUPS * GW], op_dt, name=f"op{t}")
                half = (GROUPS // 2) * chunks[t] * GW
                nc.sync.dma_start(out=op[:, 0:half], in_=op_d[t][:, 0:half])
                nc.sync.dma_start(out=op[:, half:], in_=op_d[t][:, half:])
                op_t.append(op)

            out_t = fmp.tile([ROWS, Kn], F32)
            grams = {}
            gsbs = {}
            evs = {}
            fvv = final_t[:].rearrange("p (t k c) -> p t k c", t=Kn, k=CW)

            def emit_grams(t):
                Ct = chunks[t]
                op = op_t[t]
                gram = psA.tile([128, 1024], F32, tag="gram")
                gv = gram[:].rearrange("p (g w) -> p g w", g=GROUPS)
                use_dr = cfg.get("double_row", True) and op_dt == FP8
                for g in range(GROUPS):
                    if use_dr:
                        npair = Ct // 2
                        for c in range(npair):
                            sl = op[
                                :, (g * Ct + 2 * c) * GW : (g * Ct + 2 * c + 2) * GW
                            ].rearrange("p (k w) -> p k w", k=2)
                            nc.tensor.matmul(
                                gv[0:GW, g, 0:GW], sl, sl,
                                start=(c == 0), stop=(c == npair - 1 and Ct % 2 == 0),
                                skip_group_check=True,
                                perf_mode=mybir.MatmulPerfMode.DoubleRow,
                            )
                        if Ct % 2:
                            sl = op[:, (g * Ct + Ct - 1) * GW : (g * Ct + Ct) * GW]
                            nc.tensor.matmul(
                                gv[0:GW, g, 0:GW], sl, sl,
                                start=(Ct == 1), stop=True,
                                skip_group_check=True,
                            )
                    else:
                        for c in range(Ct):
                            sl = op[:, (g * Ct + c) * GW : (g * Ct + c + 1) * GW]
                            nc.tensor.matmul(
                                gv[0:GW, g, 0:GW], sl, sl,
                                start=(c == 0), stop=(c == Ct - 1),
                                skip_group_check=True,
                            )
                grams[t] = gv
                # Act copy1 queued immediately (runs when grams stop)
                gram_sb = gsbp.tile([GW, GROUPS * GW], BF16, tag="gramsb")
                gsv = gram_sb[:].rearrange("p (g w) -> p g w", g=GROUPS)
                if t % 2 == 1:
                    nc.vector.tensor_copy(gsv[:, :, :], gv[0:GW, :, 0:GW])
                else:
                    nc.scalar.activation(gsv[:, :, :], gv[0:GW, :, 0:GW], AF.Copy)
                gsbs[t] = gsv

            def emit_selects(t):
                gsv = gsbs[t]
                ps2 = psB.tile([128, 1024], F32, tag="ps2")
                p2v = ps2[:].rearrange("p (r w) -> p r w", r=R)
                for r in range(R):
                    rhs = gsv[:, :, CW * r : CW * r + CW]
                    lhsT = sel_t[:, CW * r : CW * r + CW]
                    nc.tensor.matmul(
                        p2v[0:CW, r, 0 : GROUPS * CW], lhsT, rhs,
                        start=True, stop=True, skip_group_check=True,
                    )
                pair, tp = divmod(t, 2)
                ev = exts[pair][:].rearrange(
                    "p (r g t c) -> p r g t c", r=R, g=GROUPS, t=2
                )
                p2r = p2v[0:CW, :, 0 : GROUPS * CW].rearrange(
                    "p r (g c) -> p r g c", g=GROUPS
                )
                if t % 2 == 1:
                    nc.vector.tensor_copy(ev[:, :, :, tp, :], p2r)
                else:
                    nc.scalar.activation(ev[:, :, :, tp, :], p2r, AF.Copy)
                evs[pair] = ev

            def emit_finals(t, both=False):
                pair, tp = divmod(t, 2)
                ev = evs[pair]
                for kkc in [6, 0, 1, 2, 3, 4, 5]:
                    eng = nc.gpsimd if kkc in (2, 5) else nc.sync
                    if both:
                        eng.dma_start(
                            out=fvv[:, 2 * pair : 2 * pair + 2, kkc, :],
                            in_=ev[kkc : kkc + 1, :, :, :, :],
                        )
                    else:
                        eng.dma_start(
                            out=fvv[:, t : t + 1, kkc, :],
                            in_=ev[kkc : kkc + 1, :, :, tp : tp + 1, :],
                        )

            if do_mm and do_extract:
                emit_grams(0)
                emit_grams(1)
                emit_selects(0)
                emit_grams(2)
                emit_selects(1)
                emit_finals(1, both=True)
                emit_grams(3)
                emit_selects(2)
                emit_selects(3)
                emit_finals(3, both=True)
                if do_math:
                    fm = _FM(nc, fmp, Kn, prefix="m_")
                    _emit_math_pair(
                        nc, fm, final_t, meta_t[:], out_t[:], Kn, 0, Kn
                    )
            elif do_mm:
                for t in range(Kn):
                    emit_grams(t)
            if not (do_mm and do_extract and do_math):
                nc.vector.memset(out_t[:], 0.0)
            nc.sync.dma_start(out=out_d[:], in_=out_t[:])
    return nc


# ---------------------------------------------------------------------------
# Host side
# ---------------------------------------------------------------------------
def plan_shards(num_atoms, n_classes=K):
    """Sort rows into 32 global tiles of 128; snake-assign 4 tiles per core.

    Returns (order, assign, core_chunks): assign[c] = 4 global tile indices
    (processed big-first), core_chunks[c] = matching chunk counts.
    """
    B = num_atoms.shape[0]
    ntiles = B // ROWS
    assert ntiles == N_CORES * n_classes
    order = np.argsort(num_atoms, kind="stable")
    nas = num_atoms[order]
    tile_chunks = [
        int((int(nas[(i + 1) * ROWS - 1]) + CHUNK - 1) // CHUNK)
        for i in range(ntiles)
    ]
    assign = []
    core_chunks = []
    for c in range(N_CORES):
        tiles = [c, 15 - c, 16 + c, 31 - c]
        tiles.sort(key=lambda t: -tile_chunks[t])  # big-first
        assign.append(tiles)
        core_chunks.append([tile_chunks[t] for t in tiles])
    return order, assign, core_chunks


def _pack_tile(x, y, na, Ct):
    """x, y: [128, nmax, 3] f32 (row-major positions), na: [128] int.
    Returns op [128, Ct, GROUPS, GW] f32 with atoms on dim 0 (partitions)."""
    nmax = x.shape[1]
    cap = Ct * CHUNK
    # data [b, n, 7]
    d = np.zeros((ROWS, cap, CW), np.float32)
    ncl = min(cap, nmax)
    d[:, :ncl, 0:3] = x[:, :ncl, :]
    d[:, :ncl, 3:6] = y[:, :ncl, :]
    mask = (np.arange(cap)[None, :] < na[:, None]).astype(np.float32)
    d[:, :, 0:6] *= mask[:, :, None]
    d[:, :, 6] = 1.0
    # op[p, g, c, 7r+k] = d[8r+g, c*128+p, k]   (group-major for strip loads)
    d = d.reshape(ROWS, Ct, CHUNK, CW)            # [b, c, p, k]
    d = d.transpose(2, 1, 0, 3)                   # [p, c, b, k]
    d = d.reshape(CHUNK, Ct, R, GROUPS, CW)       # [p, c, r, g, k]  (b = 8r+g)
    d = d.transpose(0, 3, 1, 2, 4)                # [p, g, c, r, k]
    return np.ascontiguousarray(d.reshape(CHUNK, GROUPS, Ct, GW))


def _op_np_dtype():
    return mybir.dt.np(OP_DT)


def shard_inputs(coords_input, coords_target, num_atoms, order, assign, core_chunks):
    import ml_dtypes

    B, ncols = coords_input.shape
    nmax = ncols // 3
    sel = np.zeros((GW, R * CW), np.float32)
    for j in range(R * CW):
        sel[j, j] = 1.0
    sel = sel.astype(ml_dtypes.bfloat16)
    in_maps = []
    core_row_idx = []
    for c in range(N_CORES):
        m = {"sel": sel}
        idx_all = []
        Kn = len(assign[c])
        meta = np.zeros((ROWS, Kn), np.float32)
        for t in range(Kn):
            gt = assign[c][t]
            idx = order[gt * ROWS : (gt + 1) * ROWS]
            idx_all.append(idx)
            na = num_atoms[idx]
            meta[:, t] = na.astype(np.float32)
            x = coords_input[idx].reshape(ROWS, nmax, 3)
            y = coords_target[idx].reshape(ROWS, nmax, 3)
            op = _pack_tile(x, y, na, core_chunks[c][t])
            m[f"op{t}"] = np.ascontiguousarray(
                op.reshape(CHUNK, -1)
            ).astype(_op_np_dtype())
        m["meta"] = meta
        in_maps.append(m)
        core_row_idx.append(np.concatenate(idx_all))
    return in_maps, core_row_idx


def unshard_outputs(results, core_row_idx, B):
    out = np.empty(B, dtype=np.float32)
    for c in range(N_CORES):
        o = results[c]["out"]  # [ROWS, K]
        out[core_row_idx[c]] = o.T.reshape(-1)
    return out


# ---------------------------------------------------------------------------
# Entry point
# ---------------------------------------------------------------------------
_PROG_CACHE = {}


def _get_program(chunks):
    key = tuple(chunks)
    if key not in _PROG_CACHE:
        _PROG_CACHE[key] = build_program(list(chunks))
    return _PROG_CACHE[key]


def kernel(coords_input, coords_target, num_atoms):
    from concourse.bass_utils import run_bass_kernel_spmd

    x = np.ascontiguousarray(np.asarray(coords_input, dtype=np.float32))
    y = np.ascontiguousarray(np.asarray(coords_target, dtype=np.float32))
    na = np.asarray(num_atoms).astype(np.int64)
    B, ncols = x.shape
    Kn = B // (N_CORES * ROWS)
    assert B == N_CORES * ROWS * Kn, f"unsupported batch {B}"

    order, assign, core_chunks = plan_shards(na, n_classes=Kn)
    in_maps, core_row_idx = shard_inputs(x, y, na, order, assign, core_chunks)
    # group cores by identical chunk tuples -> one program per group
    groups = {}
    for c in range(N_CORES):
        groups.setdefault(tuple(core_chunks[c]), []).append(c)
    results = [None] * N_CORES
    for chunks, cores in groups.items():
        nc = _get_program(chunks)
        res = run_bass_kernel_spmd(
            nc, [in_maps[c] for c in cores], core_ids=list(range(len(cores)))
        )
        for i, c in enumerate(cores):
            results[c] = res.results[i]
    out = unshard_outputs(results, core_row_idx, B)
    return out.astype(np.float32)


# revision 35
# speedup vs baseline: 1.0104x; 1.0039x over previous
"""Bass/Trainium2 kernel for batched masked-Kabsch RMSD (nn_Coords2RMSD).

PE-centric design, one program per distinct per-core shape (8 cores):
  - Host sorts rows by num_atoms into 32 tiles of 128 rows and
    snake-assigns 4 tiles per core (big-first). Per tile, coords are
    repacked TRANSPOSED into fp8e4m3: atoms on SBUF partitions; for each
    group of 16 rows a 112-column operand [x y z X Y Z 1] per row.
    Padding atoms are zeroed on the host; the ones column makes the Gram
    matrix carry the masked sums.
  - Per (group, pair-of-128-atom-chunks) ONE symmetric DoubleRow fp8
    matmul op^T @ op accumulates in PSUM: the diagonal 7x7 blocks hold
    all 21 per-row statistics (cross-covariance, |x|^2, |y|^2, sums).
    A warmup matmul burst pins the PE pstate ramp before the real work.
  - Extraction: Act copies PSUM->SBUF (bf16), 16 identity-select
    matmuls gather the diagonal slot blocks into a second PSUM, Act
    copies them to a staging buffer, and 7 strided DMAs per tile-pair
    transpose [slot-comp, row] -> [row, stats] (sums row first so the
    final math's dependency chain can start earliest).
  - Final math on [128, K] fp32 columns: wide broadcast ops build C and
    M = C^T C; det(M - qI) comes from the characteristic-poly identity
    -2.5 q^3 + 0.5 q tr(M^2) + det(C)^2; cos(acos(r)/3 + phase) roots
    come from Newton on 4c^3 - 3c = r (cubic init, 2 iterations);
    Kabsch det sign, RMSD.
"""

import numpy as np

import concourse.bass as bass
import concourse.mybir as mybir
from concourse.tile import TileContext, ScopedClock

F32 = mybir.dt.float32
BF16 = mybir.dt.bfloat16
FP8 = mybir.dt.float8e4
OP_DT = FP8  # gram operand dtype (host-cast)
OP = mybir.AluOpType
AF = mybir.ActivationFunctionType

N_CORES = 8
ROWS = 128          # rows per tile == final partitions
GROUPS = 8          # row-groups per tile
R = 16              # rows per group
CW = 7              # cols per row: x0 x1 x2 y0 y1 y2 1
GW = R * CW         # group operand width = 112
CHUNK = 128         # atoms per matmul pass (contraction dim)
NSTAT = CW * CW     # 49 stats per row
K = 4               # classes (tiles per core)


# ---------------------------------------------------------------------------
# TileContext tail patch: this walrus build accepts at most ONE sync-wait
# command per instruction and no sem-eq waits, so the stock drain + EVSEM
# butterfly fails codegen. Emit a ge-wait-only tail instead.
# ---------------------------------------------------------------------------
def _patched_drain_and_barrier(self, tick_clock, wait_clock):
    nc = self.nc
    dummy = nc.gpsimd.nop()
    wait_clock.add_sem_waits(dummy.ins, ScopedClock({None: tick_clock.global_clock}))
    waits = list(dummy.ins.sync_info.on_wait) if dummy.ins.sync_info else []
    if dummy.ins.sync_info:
        dummy.ins.sync_info = mybir.SyncInfo(on_wait=[], on_update=[])

    bsem = nc.alloc_semaphore(f"tail_bsem_{nc.next_id()}")
    dsem = nc.alloc_semaphore(f"tail_dsem_{nc.next_id()}")
    engs = list(nc.engines.values())
    n_eng = 0
    for i, eng in enumerate(engs):
        for w in waits[i::len(engs)]:
            n = eng.nop()
            n.ins.sync_info = mybir.SyncInfo(on_wait=[w], on_update=[])
        eng.drain()
        eng.sem_inc(bsem, 1)
        n_eng += 1
    nc.gpsimd.wait_ge(bsem, n_eng)
    nc.gpsimd.sem_inc(dsem, 1)
    for eng in nc.engines.values():
        if eng is not nc.gpsimd:
            eng.wait_ge(dsem, 1)

    popped = nc._tile_sem_poison_stack.pop()
    assert popped is self._sem_poison
    nc.clear_and_free_semaphores(list(self.sems.allocated().values()))
    nc.gpsimd.sem_clear(bsem)
    nc.gpsimd.sem_clear(dsem)


def install_tile_patch():
    TileContext._drain_and_barrier = _patched_drain_and_barrier


# ---------------------------------------------------------------------------
# BIR post-pass: split multi-wait sync infos onto NoOps (walrus accepts at
# most one sync-wait command per instruction, none on Drain).
# ---------------------------------------------------------------------------
_orig_to_json_bytes = bass.Bass.to_json_bytes


def _split_multiwait_json(self) -> bytes:
    import json

    raw = _orig_to_json_bytes(self)
    m = json.loads(raw)
    ctr = 0
    changed = False
    for f in m.get("functions", []):
        for blk in f.get("blocks", []):
            insts = blk.get("instructions", [])
            out = []
            for inst in insts:
                si = inst.get("sync_info")
                ow = (si or {}).get("on_wait") or []
                opc = str(inst.get("opcode", inst.get("type", "")))
                limit = 0 if opc == "Drain" else 1
                if len(ow) > limit:
                    keep = ow[len(ow) - limit :] if limit else []
                    moved = ow[: len(ow) - limit] if limit else ow
                    for w in moved:
                        ctr += 1
                        out.append(
                            {
                                "debug": inst.get("debug", 0),
                                "engine": inst["engine"],
                                "ins": [],
                                "name": f"WS-{ctr}-{inst['name']}",
                                "opcode": "NoOp",
                                "outs": [],
                                "sync_info": {"on_update": [], "on_wait": [w]},
                            }
                        )
                    si["on_wait"] = keep
                    changed = True
                out.append(inst)
            blk["instructions"] = out
    if not changed:
        return raw
    return json.dumps(m).encode()


bass.Bass.to_json_bytes = _split_multiwait_json


# ---------------------------------------------------------------------------
# Final math emitter on [128, K] fp32 column tiles.
# final layout: [128 rows, (t: K)(kk: 7)(cc: 7)] fp32
#   G(kk, cc) = sum_n op[n, kk] op[n, cc] per row (kk,cc in 0..5 = comps,
#   6 = ones => sums). Columns for class t at offset t*49.
# ---------------------------------------------------------------------------
class _FM:
    def __init__(self, nc, pool, Kn, prefix=""):
        self.nc = nc
        self.pool = pool
        self.K = Kn
        self.n = 0
        self.prefix = prefix
        self._consts = {}

    def const_col(self, val):
        val = float(val)
        if val in self._consts:
            return self._consts[val]
        i = len(self._consts)
        t = self.pool.tile([ROWS, 1], F32, tag=f"fmc{i}", name=f"fmc{i}")
        self.nc.vector.memset(t[:], val)
        self._consts[val] = t[:]
        return t[:]

    def t(self, w=None):
        self.n += 1
        nm = f"fm{self.prefix}{self.n}"
        return self.pool.tile([ROWS, w or self.K], F32, tag=nm, name=nm)

    def tt(self, a, b, op):
        o = self.t()
        self.nc.vector.tensor_tensor(o[:], a, b, op)
        return o[:]

    def mul(self, a, b):
        return self.tt(a, b, OP.mult)

    def add(self, a, b):
        return self.tt(a, b, OP.add)

    def sub(self, a, b):
        return self.tt(a, b, OP.subtract)

    def ts(self, a, s, op):
        o = self.t()
        self.nc.vector.tensor_scalar(o[:], a, float(s), None, op)
        return o[:]

    def ts2(self, a, s1, s2, op0, op1):
        o = self.t()
        self.nc.vector.tensor_scalar(o[:], a, float(s1), float(s2), op0, op1)
        return o[:]

    def stt(self, a, s, b, op0, op1):
        """(a op0 s) op1 b"""
        o = self.t()
        self.nc.vector.scalar_tensor_tensor(o[:], a, float(s), b, op0, op1)
        return o[:]

    def act(self, a, func, bias=0.0, scale=1.0):
        o = self.t()
        if isinstance(bias, float) and bias not in (0.0, 1.0) and func != AF.Copy:
            bias = self.const_col(bias)
        self.nc.scalar.activation(o[:], a, func, bias=bias, scale=scale)
        return o[:]

    def recip(self, a):
        o = self.t()
        self.nc.vector.reciprocal(o[:], a)
        return o[:]


def _emit_math_pair(nc, fm, final_t, meta_ap, out_ap, Kn, t0, Kp):
    """Wide-op final math for classes [t0, t0+Kp)."""
    fv = final_t[:].rearrange("p (t k c) -> p t k c", t=Kn, k=CW)[
        :, t0 : t0 + Kp, :, :
    ]
    fvf = final_t[:].rearrange("p (t c) -> p t c", t=Kn)[
        :, t0 : t0 + Kp, :
    ]

    def W(w):  # fresh wide tile
        return fm.t(w)

    rn = fm.recip(meta_ap)  # [128, Kp]
    rn_b3 = rn[:, :, None].broadcast_to([ROWS, Kp, 3])

    P = fv[:, :, 0:3, 3:6]          # [128, Kp, 3, 3]
    Sall = fv[:, :, 6, 0:6]         # [128, Kp, 6]
    Sy = fv[:, :, 6, 3:6]
    rn_b6 = rn[:, :, None].broadcast_to([ROWS, Kp, 6])

    sn_t = W(Kp * 6)
    sn6 = sn_t[:].rearrange("p (t c) -> p t c", t=Kp)
    nc.vector.tensor_tensor(sn6, Sall, rn_b6, OP.mult)
    sxn = sn6[:, :, 0:3]

    t1_t = W(Kp * 9)
    t1 = t1_t[:].rearrange("p (t i j) -> p t i j", t=Kp, i=3)
    nc.vector.tensor_tensor(
        t1, sxn[:, :, :, None].broadcast_to([ROWS, Kp, 3, 3]),
        Sy[:, :, None, :].broadcast_to([ROWS, Kp, 3, 3]), OP.mult)
    C_t = W(Kp * 9)
    C = C_t[:].rearrange("p (t i j) -> p t i j", t=Kp, i=3)
    nc.vector.tensor_tensor(C, P, t1, OP.subtract)

    def Cij(i, j):
        return C[:, :, i, j]

    # M = C^T C via 3 outer products
    M_t = W(Kp * 9)
    M = M_t[:].rearrange("p (t a b) -> p t a b", t=Kp, a=3)
    tmp_t = W(Kp * 9)
    tmp = tmp_t[:].rearrange("p (t a b) -> p t a b", t=Kp, a=3)
    for i in range(3):
        Ci = C[:, :, i, :]
        dst = M if i == 0 else tmp
        nc.vector.tensor_tensor(
            dst, Ci[:, :, :, None].broadcast_to([ROWS, Kp, 3, 3]),
            Ci[:, :, None, :].broadcast_to([ROWS, Kp, 3, 3]), OP.mult)
        if i > 0:
            nc.vector.tensor_tensor(M, M, tmp, OP.add)

    Mf = M_t[:].rearrange("p (t ab) -> p t ab", t=Kp)
    Mdiag = Mf[:, :, 0:9:4]  # [128, 2, 3]

    # q = trM/3
    q = fm.add(Mdiag[:, :, 0], Mdiag[:, :, 1])
    q = fm.stt(Mdiag[:, :, 2], 1.0, q, OP.mult, OP.add)
    q = fm.ts(q, 1.0 / 3.0, OP.mult)

    # trM2 = sum M*M ; p2 = trM2 - 3 q^2
    MM_t = W(Kp * 9)
    nc.vector.tensor_tensor(MM_t[:], M_t[:], M_t[:], OP.mult)
    trM2 = fm.t()
    nc.vector.tensor_reduce(
        trM2[:], MM_t[:].rearrange("p (t ab) -> p t ab", t=Kp),
        mybir.AxisListType.X, OP.add)
    qq = fm.mul(q, q)
    p2 = fm.stt(qq, -3.0, trM2[:], OP.mult, OP.add)
    p2c = fm.ts2(p2, 1.0 / 6.0, 1e-30, OP.mult, OP.max)
    p = fm.act(p2c, AF.Sqrt)

    # --- detC, detC^2, sign (DVE; pool per-op overhead hurts the chain) ---
    def gtt(a, b, op):
        o = fm.t()
        nc.vector.tensor_tensor(o[:], a, b, op)
        return o[:]

    gm0 = gtt(Cij(1, 1), Cij(2, 2), OP.mult)
    gm0b = gtt(Cij(1, 2), Cij(2, 1), OP.mult)
    gm0 = gtt(gm0, gm0b, OP.subtract)
    gm1 = gtt(Cij(1, 0), Cij(2, 2), OP.mult)
    gm1b = gtt(Cij(1, 2), Cij(2, 0), OP.mult)
    gm1 = gtt(gm1, gm1b, OP.subtract)
    gm2 = gtt(Cij(1, 0), Cij(2, 1), OP.mult)
    gm2b = gtt(Cij(1, 1), Cij(2, 0), OP.mult)
    gm2 = gtt(gm2, gm2b, OP.subtract)
    d0 = gtt(Cij(0, 0), gm0, OP.mult)
    d1 = gtt(Cij(0, 1), gm1, OP.mult)
    d2 = gtt(Cij(0, 2), gm2, OP.mult)
    detC = gtt(gtt(d0, d1, OP.subtract), d2, OP.add)
    detC2 = gtt(detC, detC, OP.mult)
    dneg = fm.t()
    nc.vector.tensor_scalar(dneg[:], detC, 0.0, None, OP.is_lt)

    # detKq = det(M - qI) = -2.5 q^3 + 0.5 q trM2 + detC^2
    q3 = fm.mul(qq, q)
    a_ = fm.mul(q, trM2[:])
    t_ = fm.stt(a_, 0.5, detC2, OP.mult, OP.add)
    detKq = fm.stt(q3, -2.5, t_, OP.mult, OP.add)

    # r = 0.5 detKq / p^3 clamped
    rp = fm.recip(p)
    rp3 = fm.mul(fm.mul(rp, rp), rp)
    r = fm.stt(detKq, 0.5, rp3, OP.mult, OP.mult)
    r = fm.ts2(r, 1.0, -1.0, OP.min, OP.max)

    # Newton on 4c^3-3c=r for c1 (cos(phi)) and c3 (cos(phi+2pi/3)), packed
    # cubic init c1 = E(r^2) + r O(r^2); c3(r) = -c1(-r) = -E + r O
    E1, E0 = -0.07910172, 0.87011722
    O1, O0 = 0.06293734, 0.15509478
    rr = fm.mul(r, r)
    cpack_t = W(2 * Kp)
    cpack = cpack_t[:].rearrange("p (s t) -> p s t", s=2)
    Ev = fm.ts2(rr, E1, E0, OP.mult, OP.add)
    Ov = fm.ts2(rr, O1, O0, OP.mult, OP.add)
    rO = fm.mul(r, Ov)
    nc.vector.tensor_tensor(cpack[:, 0, :], Ev, rO, OP.add)
    nc.vector.tensor_tensor(cpack[:, 1, :], rO, Ev, OP.subtract)
    r_b = r[:, None, :].broadcast_to([ROWS, 2, Kp])
    for _ in range(2):
        c2 = fm.t(2 * Kp)
        nc.vector.tensor_tensor(c2[:], cpack_t[:], cpack_t[:], OP.mult)
        c3 = fm.t(2 * Kp)
        nc.vector.tensor_tensor(c3[:], c2[:], cpack_t[:], OP.mult)
        num = fm.t(2 * Kp)
        nc.vector.scalar_tensor_tensor(
            num[:].rearrange("p (s t) -> p s t", s=2),
            c3[:].rearrange("p (s t) -> p s t", s=2), 8.0, r_b,
            OP.mult, OP.add)
        den = fm.t(2 * Kp)
        nc.vector.tensor_scalar(den[:], c2[:], 12.0, -3.0, OP.mult, OP.add)
        rec = fm.t(2 * Kp)
        nc.vector.reciprocal(rec[:], den[:])
        nc.vector.tensor_tensor(cpack_t[:], num[:], rec[:], OP.mult)

    # lambdas: l1 = q + 2p c1 ; l3 = q + 2p c3 ; l2 = 3q - l1 - l3
    p2x = fm.ts(p, 2.0, OP.mult)
    lpack_t = W(3 * Kp)
    lpack = lpack_t[:].rearrange("p (s t) -> p s t", s=3)
    p2x_b = p2x[:, None, :].broadcast_to([ROWS, 2, Kp])
    q_b = q[:, None, :].broadcast_to([ROWS, 2, Kp])
    tl_t = W(2 * Kp)
    tl = tl_t[:].rearrange("p (s t) -> p s t", s=2)
    nc.vector.tensor_tensor(tl, p2x_b, cpack, OP.mult)
    nc.vector.tensor_tensor(lpack[:, 0:2, :], q_b, tl, OP.add)
    t_l2 = fm.stt(q, 3.0, lpack[:, 0, :], OP.mult, OP.subtract)
    nc.vector.tensor_tensor(lpack[:, 2, :], t_l2, lpack[:, 1, :], OP.subtract)
    lmax = fm.t(3 * Kp)
    nc.vector.tensor_scalar(lmax[:], lpack_t[:], 0.0, None, OP.max)
    spack_t = fm.t(3 * Kp)
    nc.scalar.activation(spack_t[:], lmax[:], AF.Sqrt)
    spack = spack_t[:].rearrange("p (s t) -> p s t", s=3)

    # gx + gy: one reduce over all six diag cols; packed sum-sq reduce
    Qsum = fm.t()
    nc.vector.tensor_reduce(Qsum[:], fvf[:, :, 0:41:8], mybir.AxisListType.X, OP.add)
    snS_t = W(Kp * 6)
    nc.vector.tensor_tensor(
        snS_t[:].rearrange("p (t c) -> p t c", t=Kp), sn6, Sall, OP.mult)
    s2sum = fm.t()
    nc.vector.tensor_reduce(
        s2sum[:], snS_t[:].rearrange("p (t c) -> p t c", t=Kp),
        mybir.AxisListType.X, OP.add)
    g = fm.sub(Qsum[:], s2sum[:])
    tr = fm.add(fm.add(spack[:, 0, :], spack[:, 2, :]), spack[:, 1, :])
    tr = fm.stt(fm.mul(dneg[:], spack[:, 1, :]), -2.0, tr, OP.mult, OP.add)


    diff = fm.stt(tr, -2.0, g, OP.mult, OP.add)
    msd = fm.mul(diff, rn)
    nc.scalar.activation(out_ap, fm.ts(msd, 0.0, OP.max), AF.Sqrt)


# ---------------------------------------------------------------------------
# Program builder
# ---------------------------------------------------------------------------
def build_program(chunks, cfg=None):
    """chunks: per-class chunk counts (len K). Returns nc."""
    cfg = cfg or {}
    do_mm = cfg.get("mm", True)
    do_extract = cfg.get("extract", True)
    do_math = cfg.get("math", True)
    Kn = len(chunks)
    # copy-engine parity: tiles with t % 2 == cp_par put their PSUM->SBUF
    # copies on DVE (else Act). Near-equal top tiles favor even parity.
    cp_par = 0 if (chunks[0] - chunks[1] <= 2) else 1
    install_tile_patch()
    nc = bass.Bass()
    op_dt = FP8 if cfg.get("fp8", True) else BF16
    op_d = [
        nc.dram_tensor(f"op{t}", [ROWS, chunks[t] * GROUPS * GW], op_dt,
                       kind="ExternalInput")
        for t in range(Kn)
    ]
    sel_d = nc.dram_tensor("sel", [GW, R * CW], BF16, kind="ExternalInput")
    meta_d = nc.dram_tensor("meta", [ROWS, Kn], F32, kind="ExternalInput")
    out_d = nc.dram_tensor("out", [ROWS, Kn], F32, kind="ExternalOutput")

    with TileContext(nc) as tc:
        with (
            tc.tile_pool(name="const", bufs=1) as constp,
            tc.tile_pool(name="ops", bufs=1) as opp,
            tc.tile_pool(name="gsb", bufs=2) as gsbp,
            tc.tile_pool(name="ext", bufs=1) as extp,
            tc.tile_pool(name="fmp", bufs=1) as fmp,
            tc.tile_pool(name="psA", bufs=2, space="PSUM") as psA,
            tc.tile_pool(name="psB", bufs=2, space="PSUM") as psB,
        ):
            sel_t = constp.tile([GW, R * CW], BF16)
            nc.sync.dma_start(out=sel_t[:], in_=sel_d[:])
            meta_t = constp.tile([ROWS, Kn], F32)
            nc.sync.dma_start(out=meta_t[:], in_=meta_d[:])

            # staging for rows: ext [7, (r 16)(g 8)(t 2)(c 7)] per pair
            exts = [
                extp.tile([CW, R * GROUPS * 2 * CW], F32, name=f"extp{p}")
                for p in range(Kn // 2)
            ]
            final_t = fmp.tile([ROWS, Kn * NSTAT], F32)

            # PE pstate warmup: keep PE busy during the first load so the
            # ramp to full clock completes before the first gram matmul.
            nwarm = cfg.get("warmup", 100)
            if nwarm:
                wv = psB.tile([128, 1024], F32, tag="ps2")
                for i in range(nwarm):
                    nc.tensor.matmul(
                        wv[0:CW, 0:CW], sel_t[:, 0:CW], sel_t[:, 0:CW],
                        start=True, stop=True, skip_group_check=True,
                    )

            op_t = []
            for t in range(Kn):
                op = opp.tile([ROWS, chunks[t] * GROUPS * GW], op_dt, name=f"op{t}")
                half = (GROUPS // 2) * chunks[t] * GW
                nc.sync.dma_start(out=op[:, 0:half], in_=op_d[t][:, 0:half])
                nc.sync.dma_start(out=op[:, half:], in_=op_d[t][:, half:])
                op_t.append(op)

            out_t = fmp.tile([ROWS, Kn], F32)
            grams = {}
            gsbs = {}
            evs = {}
            fvv = final_t[:].rearrange("p (t k c) -> p t k c", t=Kn, k=CW)

            def emit_grams(t):
                Ct = chunks[t]
                op = op_t[t]
                gram = psA.tile([128, 1024], F32, tag="gram")
                gv = gram[:].rearrange("p (g w) -> p g w", g=GROUPS)
                use_dr = cfg.get("double_row", True) and op_dt == FP8
                for g in range(GROUPS):
                    if use_dr:
                        npair = Ct // 2
                        for c in range(npair):
                            sl = op[
                                :, (g * Ct + 2 * c) * GW : (g * Ct + 2 * c + 2) * GW
                            ].rearrange("p (k w) -> p k w", k=2)
                            nc.tensor.matmul(
                                gv[0:GW, g, 0:GW], sl, sl,
                                start=(c == 0), stop=(c == npair - 1 and Ct % 2 == 0),
                                skip_group_check=True,
                                perf_mode=mybir.MatmulPerfMode.DoubleRow,
                            )
                        if Ct % 2:
                            sl = op[:, (g * Ct + Ct - 1) * GW : (g * Ct + Ct) * GW]
                            nc.tensor.matmul(
                                gv[0:GW, g, 0:GW], sl, sl,
                                start=(Ct == 1), stop=True,
                                skip_group_check=True,
                            )
                    else:
                        for c in range(Ct):
                            sl = op[:, (g * Ct + c) * GW : (g * Ct + c + 1) * GW]
                            nc.tensor.matmul(
                                gv[0:GW, g, 0:GW], sl, sl,
                                start=(c == 0), stop=(c == Ct - 1),
                                skip_group_check=True,
                            )
                grams[t] = gv
                # Act copy1 queued immediately (runs when grams stop)
                gram_sb = gsbp.tile([GW, GROUPS * GW], BF16, tag="gramsb")
                gsv = gram_sb[:].rearrange("p (g w) -> p g w", g=GROUPS)
                if t % 2 == cp_par:
                    nc.vector.tensor_copy(gsv[:, :, :], gv[0:GW, :, 0:GW])
                else:
                    nc.scalar.activation(gsv[:, :, :], gv[0:GW, :, 0:GW], AF.Copy)
                gsbs[t] = gsv

            def emit_selects(t):
                gsv = gsbs[t]
                ps2 = psB.tile([128, 1024], F32, tag="ps2")
                p2v = ps2[:].rearrange("p (r w) -> p r w", r=R)
                for r in range(R):
                    rhs = gsv[:, :, CW * r : CW * r + CW]
                    lhsT = sel_t[:, CW * r : CW * r + CW]
                    nc.tensor.matmul(
                        p2v[0:CW, r, 0 : GROUPS * CW], lhsT, rhs,
                        start=True, stop=True, skip_group_check=True,
                    )
                pair, tp = divmod(t, 2)
                ev = exts[pair][:].rearrange(
                    "p (r g t c) -> p r g t c", r=R, g=GROUPS, t=2
                )
                p2r = p2v[0:CW, :, 0 : GROUPS * CW].rearrange(
                    "p r (g c) -> p r g c", g=GROUPS
                )
                if t % 2 == cp_par:
                    nc.vector.tensor_copy(ev[:, :, :, tp, :], p2r)
                else:
                    nc.scalar.activation(ev[:, :, :, tp, :], p2r, AF.Copy)
                evs[pair] = ev

            def emit_finals(t, both=False):
                pair, tp = divmod(t, 2)
                ev = evs[pair]
                for kkc in [6, 0, 1, 2, 3, 4, 5]:
                    eng = nc.gpsimd if kkc in (2, 5) else nc.sync
                    if both:
                        eng.dma_start(
                            out=fvv[:, 2 * pair : 2 * pair + 2, kkc, :],
                            in_=ev[kkc : kkc + 1, :, :, :, :],
                        )
                    else:
                        eng.dma_start(
                            out=fvv[:, t : t + 1, kkc, :],
                            in_=ev[kkc : kkc + 1, :, :, tp : tp + 1, :],
                        )

            if do_mm and do_extract:
                emit_grams(0)
                emit_grams(1)
                emit_selects(0)
                emit_grams(2)
                emit_selects(1)
                emit_finals(1, both=True)
                emit_grams(3)
                emit_selects(2)
                emit_selects(3)
                emit_finals(3, both=True)
                if do_math:
                    fm = _FM(nc, fmp, Kn, prefix="m_")
                    _emit_math_pair(
                        nc, fm, final_t, meta_t[:], out_t[:], Kn, 0, Kn
                    )
            elif do_mm:
                for t in range(Kn):
                    emit_grams(t)
            if not (do_mm and do_extract and do_math):
                nc.vector.memset(out_t[:], 0.0)
            nc.sync.dma_start(out=out_d[:], in_=out_t[:])
    return nc


# ---------------------------------------------------------------------------
# Host side
# ---------------------------------------------------------------------------
def plan_shards(num_atoms, n_classes=K):
    """Sort rows into 32 global tiles of 128; snake-assign 4 tiles per core.

    Returns (order, assign, core_chunks): assign[c] = 4 global tile indices
    (processed big-first), core_chunks[c] = matching chunk counts.
    """
    B = num_atoms.shape[0]
    ntiles = B // ROWS
    assert ntiles == N_CORES * n_classes
    order = np.argsort(num_atoms, kind="stable")
    nas = num_atoms[order]
    tile_chunks = [
        int((int(nas[(i + 1) * ROWS - 1]) + CHUNK - 1) // CHUNK)
        for i in range(ntiles)
    ]
    assign = []
    core_chunks = []
    for c in range(N_CORES):
        tiles = [c, 15 - c, 16 + c, 31 - c]
        tiles.sort(key=lambda t: -tile_chunks[t])  # big-first
        assign.append(tiles)
        core_chunks.append([tile_chunks[t] for t in tiles])
    return order, assign, core_chunks


def _pack_tile(x, y, na, Ct):
    """x, y: [128, nmax, 3] f32 (row-major positions), na: [128] int.
    Returns op [128, Ct, GROUPS, GW] f32 with atoms on dim 0 (partitions)."""
    nmax = x.shape[1]
    cap = Ct * CHUNK
    # data [b, n, 7]
    d = np.zeros((ROWS, cap, CW), np.float32)
    ncl = min(cap, nmax)
    d[:, :ncl, 0:3] = x[:, :ncl, :]
    d[:, :ncl, 3:6] = y[:, :ncl, :]
    mask = (np.arange(cap)[None, :] < na[:, None]).astype(np.float32)
    d[:, :, 0:6] *= mask[:, :, None]
    d[:, :, 6] = 1.0
    # op[p, g, c, 7r+k] = d[8r+g, c*128+p, k]   (group-major for strip loads)
    d = d.reshape(ROWS, Ct, CHUNK, CW)            # [b, c, p, k]
    d = d.transpose(2, 1, 0, 3)                   # [p, c, b, k]
    d = d.reshape(CHUNK, Ct, R, GROUPS, CW)       # [p, c, r, g, k]  (b = 8r+g)
    d = d.transpose(0, 3, 1, 2, 4)                # [p, g, c, r, k]
    return np.ascontiguousarray(d.reshape(CHUNK, GROUPS, Ct, GW))


def _op_np_dtype():
    return mybir.dt.np(OP_DT)


def shard_inputs(coords_input, coords_target, num_atoms, order, assign, core_chunks):
    import ml_dtypes

    B, ncols = coords_input.shape
    nmax = ncols // 3
    sel = np.zeros((GW, R * CW), np.float32)
    for j in range(R * CW):
        sel[j, j] = 1.0
    sel = sel.astype(ml_dtypes.bfloat16)
    in_maps = []
    core_row_idx = []
    for c in range(N_CORES):
        m = {"sel": sel}
        idx_all = []
        Kn = len(assign[c])
        meta = np.zeros((ROWS, Kn), np.float32)
        for t in range(Kn):
            gt = assign[c][t]
            idx = order[gt * ROWS : (gt + 1) * ROWS]
            idx_all.append(idx)
            na = num_atoms[idx]
            meta[:, t] = na.astype(np.float32)
            x = coords_input[idx].reshape(ROWS, nmax, 3)
            y = coords_target[idx].reshape(ROWS, nmax, 3)
            op = _pack_tile(x, y, na, core_chunks[c][t])
            m[f"op{t}"] = np.ascontiguousarray(
                op.reshape(CHUNK, -1)
            ).astype(_op_np_dtype())
        m["meta"] = meta
        in_maps.append(m)
        core_row_idx.append(np.concatenate(idx_all))
    return in_maps, core_row_idx


def unshard_outputs(results, core_row_idx, B):
    out = np.empty(B, dtype=np.float32)
    for c in range(N_CORES):
        o = results[c]["out"]  # [ROWS, K]
        out[core_row_idx[c]] = o.T.reshape(-1)
    return out


# ---------------------------------------------------------------------------
# Entry point
# ---------------------------------------------------------------------------
_PROG_CACHE = {}


def _get_program(chunks):
    key = tuple(chunks)
    if key not in _PROG_CACHE:
        _PROG_CACHE[key] = build_program(list(chunks))
    return _PROG_CACHE[key]


def kernel(coords_input, coords_target, num_atoms):
    from concourse.bass_utils import run_bass_kernel_spmd

    x = np.ascontiguousarray(np.asarray(coords_input, dtype=np.float32))
    y = np.ascontiguousarray(np.asarray(coords_target, dtype=np.float32))
    na = np.asarray(num_atoms).astype(np.int64)
    B, ncols = x.shape
    Kn = B // (N_CORES * ROWS)
    assert B == N_CORES * ROWS * Kn, f"unsupported batch {B}"

    order, assign, core_chunks = plan_shards(na, n_classes=Kn)
    in_maps, core_row_idx = shard_inputs(x, y, na, order, assign, core_chunks)
    # group cores by identical chunk tuples -> one program per group
    groups = {}
    for c in range(N_CORES):
        groups.setdefault(tuple(core_chunks[c]), []).append(c)
    results = [None] * N_CORES
    for chunks, cores in groups.items():
        nc = _get_program(chunks)
        res = run_bass_kernel_spmd(
            nc, [in_maps[c] for c in cores], core_ids=list(range(len(cores)))
        )
        for i, c in enumerate(cores):
            results[c] = res.results[i]
    out = unshard_outputs(results, core_row_idx, B)
    return out.astype(np.float32)


# revision 36
# speedup vs baseline: 1.0180x; 1.0076x over previous
"""Bass/Trainium2 kernel for batched masked-Kabsch RMSD (nn_Coords2RMSD).

PE-centric design, one program per distinct per-core shape (8 cores):
  - Host sorts rows by num_atoms into 32 tiles of 128 rows and
    snake-assigns 4 tiles per core (big-first). Per tile, coords are
    repacked TRANSPOSED into fp8e4m3: atoms on SBUF partitions; for each
    group of 16 rows a 112-column operand [x y z X Y Z 1] per row.
    Padding atoms are zeroed on the host; the ones column makes the Gram
    matrix carry the masked sums.
  - Per (group, pair-of-128-atom-chunks) ONE symmetric DoubleRow fp8
    matmul op^T @ op accumulates in PSUM: the diagonal 7x7 blocks hold
    all 21 per-row statistics (cross-covariance, |x|^2, |y|^2, sums).
    A warmup matmul burst pins the PE pstate ramp before the real work.
  - Extraction: Act copies PSUM->SBUF (bf16), 16 identity-select
    matmuls gather the diagonal slot blocks into a second PSUM, Act
    copies them to a staging buffer, and 7 strided DMAs per tile-pair
    transpose [slot-comp, row] -> [row, stats] (sums row first so the
    final math's dependency chain can start earliest).
  - Final math on [128, K] fp32 columns: wide broadcast ops build C and
    M = C^T C; det(M - qI) comes from the characteristic-poly identity
    -2.5 q^3 + 0.5 q tr(M^2) + det(C)^2; cos(acos(r)/3 + phase) roots
    come from Newton on 4c^3 - 3c = r (cubic init, 2 iterations);
    Kabsch det sign, RMSD.
"""

import numpy as np

import concourse.bass as bass
import concourse.mybir as mybir
from concourse.tile import TileContext, ScopedClock

F32 = mybir.dt.float32
BF16 = mybir.dt.bfloat16
FP8 = mybir.dt.float8e4
OP_DT = FP8  # gram operand dtype (host-cast)
OP = mybir.AluOpType
AF = mybir.ActivationFunctionType

N_CORES = 8
ROWS = 128          # rows per tile == final partitions
GROUPS = 8          # row-groups per tile
R = 16              # rows per group
CW = 7              # cols per row: x0 x1 x2 y0 y1 y2 1
GW = R * CW         # group operand width = 112
CHUNK = 128         # atoms per matmul pass (contraction dim)
NSTAT = CW * CW     # 49 stats per row
K = 4               # classes (tiles per core)


# ---------------------------------------------------------------------------
# TileContext tail patch: this walrus build accepts at most ONE sync-wait
# command per instruction and no sem-eq waits, so the stock drain + EVSEM
# butterfly fails codegen. Emit a ge-wait-only tail instead.
# ---------------------------------------------------------------------------
def _patched_drain_and_barrier(self, tick_clock, wait_clock):
    nc = self.nc
    dummy = nc.gpsimd.nop()
    wait_clock.add_sem_waits(dummy.ins, ScopedClock({None: tick_clock.global_clock}))
    waits = list(dummy.ins.sync_info.on_wait) if dummy.ins.sync_info else []
    if dummy.ins.sync_info:
        dummy.ins.sync_info = mybir.SyncInfo(on_wait=[], on_update=[])

    bsem = nc.alloc_semaphore(f"tail_bsem_{nc.next_id()}")
    dsem = nc.alloc_semaphore(f"tail_dsem_{nc.next_id()}")
    engs = list(nc.engines.values())
    n_eng = 0
    for i, eng in enumerate(engs):
        for w in waits[i::len(engs)]:
            n = eng.nop()
            n.ins.sync_info = mybir.SyncInfo(on_wait=[w], on_update=[])
        eng.drain()
        eng.sem_inc(bsem, 1)
        n_eng += 1
    nc.gpsimd.wait_ge(bsem, n_eng)
    nc.gpsimd.sem_inc(dsem, 1)
    for eng in nc.engines.values():
        if eng is not nc.gpsimd:
            eng.wait_ge(dsem, 1)

    popped = nc._tile_sem_poison_stack.pop()
    assert popped is self._sem_poison
    nc.clear_and_free_semaphores(list(self.sems.allocated().values()))
    nc.gpsimd.sem_clear(bsem)
    nc.gpsimd.sem_clear(dsem)


def install_tile_patch():
    TileContext._drain_and_barrier = _patched_drain_and_barrier


# ---------------------------------------------------------------------------
# BIR post-pass: split multi-wait sync infos onto NoOps (walrus accepts at
# most one sync-wait command per instruction, none on Drain).
# ---------------------------------------------------------------------------
_orig_to_json_bytes = bass.Bass.to_json_bytes


def _split_multiwait_json(self) -> bytes:
    import json

    raw = _orig_to_json_bytes(self)
    m = json.loads(raw)
    ctr = 0
    changed = False
    for f in m.get("functions", []):
        for blk in f.get("blocks", []):
            insts = blk.get("instructions", [])
            out = []
            for inst in insts:
                si = inst.get("sync_info")
                ow = (si or {}).get("on_wait") or []
                opc = str(inst.get("opcode", inst.get("type", "")))
                limit = 0 if opc == "Drain" else 1
                if len(ow) > limit:
                    keep = ow[len(ow) - limit :] if limit else []
                    moved = ow[: len(ow) - limit] if limit else ow
                    for w in moved:
                        ctr += 1
                        out.append(
                            {
                                "debug": inst.get("debug", 0),
                                "engine": inst["engine"],
                                "ins": [],
                                "name": f"WS-{ctr}-{inst['name']}",
                                "opcode": "NoOp",
                                "outs": [],
                                "sync_info": {"on_update": [], "on_wait": [w]},
                            }
                        )
                    si["on_wait"] = keep
                    changed = True
                out.append(inst)
            blk["instructions"] = out
    if not changed:
        return raw
    return json.dumps(m).encode()


bass.Bass.to_json_bytes = _split_multiwait_json


# ---------------------------------------------------------------------------
# Final math emitter on [128, K] fp32 column tiles.
# final layout: [128 rows, (t: K)(kk: 7)(cc: 7)] fp32
#   G(kk, cc) = sum_n op[n, kk] op[n, cc] per row (kk,cc in 0..5 = comps,
#   6 = ones => sums). Columns for class t at offset t*49.
# ---------------------------------------------------------------------------
class _FM:
    def __init__(self, nc, pool, Kn, prefix=""):
        self.nc = nc
        self.pool = pool
        self.K = Kn
        self.n = 0
        self.prefix = prefix
        self._consts = {}

    def const_col(self, val):
        val = float(val)
        if val in self._consts:
            return self._consts[val]
        i = len(self._consts)
        t = self.pool.tile([ROWS, 1], F32, tag=f"fmc{i}", name=f"fmc{i}")
        self.nc.vector.memset(t[:], val)
        self._consts[val] = t[:]
        return t[:]

    def t(self, w=None):
        self.n += 1
        nm = f"fm{self.prefix}{self.n}"
        return self.pool.tile([ROWS, w or self.K], F32, tag=nm, name=nm)

    def tt(self, a, b, op):
        o = self.t()
        self.nc.vector.tensor_tensor(o[:], a, b, op)
        return o[:]

    def mul(self, a, b):
        return self.tt(a, b, OP.mult)

    def add(self, a, b):
        return self.tt(a, b, OP.add)

    def sub(self, a, b):
        return self.tt(a, b, OP.subtract)

    def ts(self, a, s, op):
        o = self.t()
        self.nc.vector.tensor_scalar(o[:], a, float(s), None, op)
        return o[:]

    def ts2(self, a, s1, s2, op0, op1):
        o = self.t()
        self.nc.vector.tensor_scalar(o[:], a, float(s1), float(s2), op0, op1)
        return o[:]

    def stt(self, a, s, b, op0, op1):
        """(a op0 s) op1 b"""
        o = self.t()
        self.nc.vector.scalar_tensor_tensor(o[:], a, float(s), b, op0, op1)
        return o[:]

    def act(self, a, func, bias=0.0, scale=1.0):
        o = self.t()
        if isinstance(bias, float) and bias not in (0.0, 1.0) and func != AF.Copy:
            bias = self.const_col(bias)
        self.nc.scalar.activation(o[:], a, func, bias=bias, scale=scale)
        return o[:]

    def recip(self, a):
        o = self.t()
        self.nc.vector.reciprocal(o[:], a)
        return o[:]


def _emit_math_pair(nc, fm, final_t, meta_ap, out_ap, Kn, t0, Kp):
    """Wide-op final math for classes [t0, t0+Kp)."""
    fv = final_t[:].rearrange("p (t k c) -> p t k c", t=Kn, k=CW)[
        :, t0 : t0 + Kp, :, :
    ]
    fvf = final_t[:].rearrange("p (t c) -> p t c", t=Kn)[
        :, t0 : t0 + Kp, :
    ]

    def W(w):  # fresh wide tile
        return fm.t(w)

    rn = fm.recip(meta_ap)  # [128, Kp]
    rn_b3 = rn[:, :, None].broadcast_to([ROWS, Kp, 3])

    P = fv[:, :, 0:3, 3:6]          # [128, Kp, 3, 3]
    Sall = fv[:, :, 6, 0:6]         # [128, Kp, 6]
    Sy = fv[:, :, 6, 3:6]
    rn_b6 = rn[:, :, None].broadcast_to([ROWS, Kp, 6])

    sn_t = W(Kp * 6)
    sn6 = sn_t[:].rearrange("p (t c) -> p t c", t=Kp)
    nc.vector.tensor_tensor(sn6, Sall, rn_b6, OP.mult)
    sxn = sn6[:, :, 0:3]

    t1_t = W(Kp * 9)
    t1 = t1_t[:].rearrange("p (t i j) -> p t i j", t=Kp, i=3)
    nc.vector.tensor_tensor(
        t1, sxn[:, :, :, None].broadcast_to([ROWS, Kp, 3, 3]),
        Sy[:, :, None, :].broadcast_to([ROWS, Kp, 3, 3]), OP.mult)
    C_t = W(Kp * 9)
    C = C_t[:].rearrange("p (t i j) -> p t i j", t=Kp, i=3)
    nc.vector.tensor_tensor(C, P, t1, OP.subtract)

    def Cij(i, j):
        return C[:, :, i, j]

    # M = C^T C via 3 outer products
    M_t = W(Kp * 9)
    M = M_t[:].rearrange("p (t a b) -> p t a b", t=Kp, a=3)
    tmp_t = W(Kp * 9)
    tmp = tmp_t[:].rearrange("p (t a b) -> p t a b", t=Kp, a=3)
    for i in range(3):
        Ci = C[:, :, i, :]
        dst = M if i == 0 else tmp
        nc.vector.tensor_tensor(
            dst, Ci[:, :, :, None].broadcast_to([ROWS, Kp, 3, 3]),
            Ci[:, :, None, :].broadcast_to([ROWS, Kp, 3, 3]), OP.mult)
        if i > 0:
            nc.vector.tensor_tensor(M, M, tmp, OP.add)

    Mf = M_t[:].rearrange("p (t ab) -> p t ab", t=Kp)
    Mdiag = Mf[:, :, 0:9:4]  # [128, 2, 3]

    # q = trM/3
    q = fm.add(Mdiag[:, :, 0], Mdiag[:, :, 1])
    q = fm.stt(Mdiag[:, :, 2], 1.0, q, OP.mult, OP.add)
    q = fm.ts(q, 1.0 / 3.0, OP.mult)

    # trM2 = sum M*M ; p2 = trM2 - 3 q^2
    MM_t = W(Kp * 9)
    nc.vector.tensor_tensor(MM_t[:], M_t[:], M_t[:], OP.mult)
    trM2 = fm.t()
    nc.vector.tensor_reduce(
        trM2[:], MM_t[:].rearrange("p (t ab) -> p t ab", t=Kp),
        mybir.AxisListType.X, OP.add)
    qq = fm.mul(q, q)
    p2 = fm.stt(qq, -3.0, trM2[:], OP.mult, OP.add)
    p2c = fm.ts2(p2, 1.0 / 6.0, 1e-30, OP.mult, OP.max)
    p = fm.act(p2c, AF.Sqrt)

    # --- detC, detC^2, sign (DVE; pool per-op overhead hurts the chain) ---
    def gtt(a, b, op):
        o = fm.t()
        nc.vector.tensor_tensor(o[:], a, b, op)
        return o[:]

    gm0 = gtt(Cij(1, 1), Cij(2, 2), OP.mult)
    gm0b = gtt(Cij(1, 2), Cij(2, 1), OP.mult)
    gm0 = gtt(gm0, gm0b, OP.subtract)
    gm1 = gtt(Cij(1, 0), Cij(2, 2), OP.mult)
    gm1b = gtt(Cij(1, 2), Cij(2, 0), OP.mult)
    gm1 = gtt(gm1, gm1b, OP.subtract)
    gm2 = gtt(Cij(1, 0), Cij(2, 1), OP.mult)
    gm2b = gtt(Cij(1, 1), Cij(2, 0), OP.mult)
    gm2 = gtt(gm2, gm2b, OP.subtract)
    d0 = gtt(Cij(0, 0), gm0, OP.mult)
    d1 = gtt(Cij(0, 1), gm1, OP.mult)
    d2 = gtt(Cij(0, 2), gm2, OP.mult)
    detC = gtt(gtt(d0, d1, OP.subtract), d2, OP.add)
    detC2 = gtt(detC, detC, OP.mult)
    dneg = fm.t()
    nc.vector.tensor_scalar(dneg[:], detC, 0.0, None, OP.is_lt)

    # detKq = det(M - qI) = -2.5 q^3 + 0.5 q trM2 + detC^2
    q3 = fm.mul(qq, q)
    a_ = fm.mul(q, trM2[:])
    t_ = fm.stt(a_, 0.5, detC2, OP.mult, OP.add)
    detKq = fm.stt(q3, -2.5, t_, OP.mult, OP.add)

    # r = 0.5 detKq / p^3 clamped
    rp = fm.recip(p)
    rp3 = fm.mul(fm.mul(rp, rp), rp)
    r = fm.stt(detKq, 0.5, rp3, OP.mult, OP.mult)
    r = fm.ts2(r, 1.0, -1.0, OP.min, OP.max)

    # Newton on 4c^3-3c=r for c1 (cos(phi)) and c3 (cos(phi+2pi/3)), packed
    # cubic init c1 = E(r^2) + r O(r^2); c3(r) = -c1(-r) = -E + r O
    E1, E0 = -0.07910172, 0.87011722
    O1, O0 = 0.06293734, 0.15509478
    rr = fm.mul(r, r)
    cpack_t = W(2 * Kp)
    cpack = cpack_t[:].rearrange("p (s t) -> p s t", s=2)
    Ev = fm.ts2(rr, E1, E0, OP.mult, OP.add)
    Ov = fm.ts2(rr, O1, O0, OP.mult, OP.add)
    rO = fm.mul(r, Ov)
    nc.vector.tensor_tensor(cpack[:, 0, :], Ev, rO, OP.add)
    nc.vector.tensor_tensor(cpack[:, 1, :], rO, Ev, OP.subtract)
    r_b = r[:, None, :].broadcast_to([ROWS, 2, Kp])
    for _ in range(2):
        c2 = fm.t(2 * Kp)
        nc.vector.tensor_tensor(c2[:], cpack_t[:], cpack_t[:], OP.mult)
        c3 = fm.t(2 * Kp)
        nc.vector.tensor_tensor(c3[:], c2[:], cpack_t[:], OP.mult)
        num = fm.t(2 * Kp)
        nc.vector.scalar_tensor_tensor(
            num[:].rearrange("p (s t) -> p s t", s=2),
            c3[:].rearrange("p (s t) -> p s t", s=2), 8.0, r_b,
            OP.mult, OP.add)
        den = fm.t(2 * Kp)
        nc.vector.tensor_scalar(den[:], c2[:], 12.0, -3.0, OP.mult, OP.add)
        rec = fm.t(2 * Kp)
        nc.vector.reciprocal(rec[:], den[:])
        nc.vector.tensor_tensor(cpack_t[:], num[:], rec[:], OP.mult)

    # lambdas: l1 = q + 2p c1 ; l3 = q + 2p c3 ; l2 = 3q - l1 - l3
    p2x = fm.ts(p, 2.0, OP.mult)
    lpack_t = W(3 * Kp)
    lpack = lpack_t[:].rearrange("p (s t) -> p s t", s=3)
    p2x_b = p2x[:, None, :].broadcast_to([ROWS, 2, Kp])
    q_b = q[:, None, :].broadcast_to([ROWS, 2, Kp])
    tl_t = W(2 * Kp)
    tl = tl_t[:].rearrange("p (s t) -> p s t", s=2)
    nc.vector.tensor_tensor(tl, p2x_b, cpack, OP.mult)
    nc.vector.tensor_tensor(lpack[:, 0:2, :], q_b, tl, OP.add)
    t_l2 = fm.stt(q, 3.0, lpack[:, 0, :], OP.mult, OP.subtract)
    nc.vector.tensor_tensor(lpack[:, 2, :], t_l2, lpack[:, 1, :], OP.subtract)
    lmax = fm.t(3 * Kp)
    nc.vector.tensor_scalar(lmax[:], lpack_t[:], 0.0, None, OP.max)
    spack_t = fm.t(3 * Kp)
    nc.scalar.activation(spack_t[:], lmax[:], AF.Sqrt)
    spack = spack_t[:].rearrange("p (s t) -> p s t", s=3)

    # gx + gy: one reduce over all six diag cols; packed sum-sq reduce
    Qsum = fm.t()
    nc.vector.tensor_reduce(Qsum[:], fvf[:, :, 0:41:8], mybir.AxisListType.X, OP.add)
    snS_t = W(Kp * 6)
    nc.vector.tensor_tensor(
        snS_t[:].rearrange("p (t c) -> p t c", t=Kp), sn6, Sall, OP.mult)
    s2sum = fm.t()
    nc.vector.tensor_reduce(
        s2sum[:], snS_t[:].rearrange("p (t c) -> p t c", t=Kp),
        mybir.AxisListType.X, OP.add)
    g = fm.sub(Qsum[:], s2sum[:])
    tr = fm.add(fm.add(spack[:, 0, :], spack[:, 2, :]), spack[:, 1, :])
    tr = fm.stt(fm.mul(dneg[:], spack[:, 1, :]), -2.0, tr, OP.mult, OP.add)


    diff = fm.stt(tr, -2.0, g, OP.mult, OP.add)
    msd = fm.mul(diff, rn)
    nc.scalar.activation(out_ap, fm.ts(msd, 0.0, OP.max), AF.Sqrt)


# ---------------------------------------------------------------------------
# Program builder
# ---------------------------------------------------------------------------
def build_program(chunks, cfg=None):
    """chunks: per-class chunk counts (len K). Returns nc."""
    cfg = cfg or {}
    do_mm = cfg.get("mm", True)
    do_extract = cfg.get("extract", True)
    do_math = cfg.get("math", True)
    Kn = len(chunks)
    # copy-engine parity: tiles with t % 2 == cp_par put their PSUM->SBUF
    # copies on DVE (else Act). Near-equal top tiles favor even parity.
    cp_par = 0 if (chunks[0] - chunks[1] <= 2) else 1
    install_tile_patch()
    nc = bass.Bass()
    op_dt = FP8 if cfg.get("fp8", True) else BF16
    op_d = [
        nc.dram_tensor(f"op{t}", [ROWS, chunks[t] * GROUPS * GW], op_dt,
                       kind="ExternalInput")
        for t in range(Kn)
    ]
    sel_d = nc.dram_tensor("sel", [GW, R * CW], BF16, kind="ExternalInput")
    meta_d = nc.dram_tensor("meta", [ROWS, Kn], F32, kind="ExternalInput")
    out_d = nc.dram_tensor("out", [ROWS, Kn], F32, kind="ExternalOutput")

    with TileContext(nc) as tc:
        with (
            tc.tile_pool(name="const", bufs=1) as constp,
            tc.tile_pool(name="ops", bufs=1) as opp,
            tc.tile_pool(name="gsb", bufs=2) as gsbp,
            tc.tile_pool(name="ext", bufs=1) as extp,
            tc.tile_pool(name="fmp", bufs=1) as fmp,
            tc.tile_pool(name="psA", bufs=2, space="PSUM") as psA,
            tc.tile_pool(name="psB", bufs=2, space="PSUM") as psB,
        ):
            sel_t = constp.tile([GW, R * CW], BF16)
            nc.sync.dma_start(out=sel_t[:], in_=sel_d[:])
            meta_t = constp.tile([ROWS, Kn], F32)
            nc.sync.dma_start(out=meta_t[:], in_=meta_d[:])

            # staging for rows: ext [7, (r 16)(g 8)(t 2)(c 7)] per pair
            exts = [extp.tile([CW, R * GROUPS * Kn * CW], F32, name="extall")]
            final_t = fmp.tile([ROWS, Kn * NSTAT], F32)

            # PE pstate warmup: keep PE busy during the first load so the
            # ramp to full clock completes before the first gram matmul.
            nwarm = cfg.get("warmup", 100)
            if nwarm:
                wv = psB.tile([128, 1024], F32, tag="ps2")
                for i in range(nwarm):
                    nc.tensor.matmul(
                        wv[0:CW, 0:CW], sel_t[:, 0:CW], sel_t[:, 0:CW],
                        start=True, stop=True, skip_group_check=True,
                    )

            op_t = []
            for t in range(Kn):
                op = opp.tile([ROWS, chunks[t] * GROUPS * GW], op_dt, name=f"op{t}")
                half = (GROUPS // 2) * chunks[t] * GW
                nc.sync.dma_start(out=op[:, 0:half], in_=op_d[t][:, 0:half])
                nc.sync.dma_start(out=op[:, half:], in_=op_d[t][:, half:])
                op_t.append(op)

            out_t = fmp.tile([ROWS, Kn], F32)
            grams = {}
            gsbs = {}
            evs = {}
            fvv = final_t[:].rearrange("p (t k c) -> p t k c", t=Kn, k=CW)

            def emit_grams(t):
                Ct = chunks[t]
                op = op_t[t]
                gram = psA.tile([128, 1024], F32, tag="gram")
                gv = gram[:].rearrange("p (g w) -> p g w", g=GROUPS)
                use_dr = cfg.get("double_row", True) and op_dt == FP8
                for g in range(GROUPS):
                    if use_dr:
                        npair = Ct // 2
                        for c in range(npair):
                            sl = op[
                                :, (g * Ct + 2 * c) * GW : (g * Ct + 2 * c + 2) * GW
                            ].rearrange("p (k w) -> p k w", k=2)
                            nc.tensor.matmul(
                                gv[0:GW, g, 0:GW], sl, sl,
                                start=(c == 0), stop=(c == npair - 1 and Ct % 2 == 0),
                                skip_group_check=True,
                                perf_mode=mybir.MatmulPerfMode.DoubleRow,
                            )
                        if Ct % 2:
                            sl = op[:, (g * Ct + Ct - 1) * GW : (g * Ct + Ct) * GW]
                            nc.tensor.matmul(
                                gv[0:GW, g, 0:GW], sl, sl,
                                start=(Ct == 1), stop=True,
                                skip_group_check=True,
                            )
                    else:
                        for c in range(Ct):
                            sl = op[:, (g * Ct + c) * GW : (g * Ct + c + 1) * GW]
                            nc.tensor.matmul(
                                gv[0:GW, g, 0:GW], sl, sl,
                                start=(c == 0), stop=(c == Ct - 1),
                                skip_group_check=True,
                            )
                grams[t] = gv
                # Act copy1 queued immediately (runs when grams stop)
                gram_sb = gsbp.tile([GW, GROUPS * GW], BF16, tag="gramsb")
                gsv = gram_sb[:].rearrange("p (g w) -> p g w", g=GROUPS)
                if t % 2 == cp_par:
                    nc.vector.tensor_copy(gsv[:, :, :], gv[0:GW, :, 0:GW])
                else:
                    nc.scalar.activation(gsv[:, :, :], gv[0:GW, :, 0:GW], AF.Copy)
                gsbs[t] = gsv

            def emit_selects(t):
                gsv = gsbs[t]
                ps2 = psB.tile([128, 1024], F32, tag="ps2")
                p2v = ps2[:].rearrange("p (r w) -> p r w", r=R)
                for r in range(R):
                    rhs = gsv[:, :, CW * r : CW * r + CW]
                    lhsT = sel_t[:, CW * r : CW * r + CW]
                    nc.tensor.matmul(
                        p2v[0:CW, r, 0 : GROUPS * CW], lhsT, rhs,
                        start=True, stop=True, skip_group_check=True,
                    )
                pair, tp = 0, t
                ev = exts[0][:].rearrange(
                    "p (r g t c) -> p r g t c", r=R, g=GROUPS, t=Kn
                )
                p2r = p2v[0:CW, :, 0 : GROUPS * CW].rearrange(
                    "p r (g c) -> p r g c", g=GROUPS
                )
                if t % 2 == cp_par:
                    nc.vector.tensor_copy(ev[:, :, :, tp, :], p2r)
                else:
                    nc.scalar.activation(ev[:, :, :, tp, :], p2r, AF.Copy)
                evs[pair] = ev

            def emit_finals(t, both=False):
                # one staging tile for all classes: 7 transpose DMAs total
                ev = evs[0]
                for kkc in [6, 0, 1, 2, 3, 4, 5]:
                    eng = nc.gpsimd if kkc in (2, 5) else nc.sync
                    eng.dma_start(
                        out=fvv[:, :, kkc, :],
                        in_=ev[kkc : kkc + 1, :, :, :, :],
                    )

            if do_mm and do_extract:
                emit_grams(0)
                emit_grams(1)
                emit_selects(0)
                emit_grams(2)
                emit_selects(1)
                emit_grams(3)
                emit_selects(2)
                emit_selects(3)
                emit_finals(3)
                if do_math:
                    fm = _FM(nc, fmp, Kn, prefix="m_")
                    _emit_math_pair(
                        nc, fm, final_t, meta_t[:], out_t[:], Kn, 0, Kn
                    )
            elif do_mm:
                for t in range(Kn):
                    emit_grams(t)
            if not (do_mm and do_extract and do_math):
                nc.vector.memset(out_t[:], 0.0)
            nc.sync.dma_start(out=out_d[:], in_=out_t[:])
    return nc


# ---------------------------------------------------------------------------
# Host side
# ---------------------------------------------------------------------------
def plan_shards(num_atoms, n_classes=K):
    """Sort rows into 32 global tiles of 128; snake-assign 4 tiles per core.

    Returns (order, assign, core_chunks): assign[c] = 4 global tile indices
    (processed big-first), core_chunks[c] = matching chunk counts.
    """
    B = num_atoms.shape[0]
    ntiles = B // ROWS
    assert ntiles == N_CORES * n_classes
    order = np.argsort(num_atoms, kind="stable")
    nas = num_atoms[order]
    tile_chunks = [
        int((int(nas[(i + 1) * ROWS - 1]) + CHUNK - 1) // CHUNK)
        for i in range(ntiles)
    ]
    assign = []
    core_chunks = []
    for c in range(N_CORES):
        tiles = [c, 15 - c, 16 + c, 31 - c]
        tiles.sort(key=lambda t: -tile_chunks[t])  # big-first
        assign.append(tiles)
        core_chunks.append([tile_chunks[t] for t in tiles])
    return order, assign, core_chunks


def _pack_tile(x, y, na, Ct):
    """x, y: [128, nmax, 3] f32 (row-major positions), na: [128] int.
    Returns op [128, Ct, GROUPS, GW] f32 with atoms on dim 0 (partitions)."""
    nmax = x.shape[1]
    cap = Ct * CHUNK
    # data [b, n, 7]
    d = np.zeros((ROWS, cap, CW), np.float32)
    ncl = min(cap, nmax)
    d[:, :ncl, 0:3] = x[:, :ncl, :]
    d[:, :ncl, 3:6] = y[:, :ncl, :]
    mask = (np.arange(cap)[None, :] < na[:, None]).astype(np.float32)
    d[:, :, 0:6] *= mask[:, :, None]
    d[:, :, 6] = 1.0
    # op[p, g, c, 7r+k] = d[8r+g, c*128+p, k]   (group-major for strip loads)
    d = d.reshape(ROWS, Ct, CHUNK, CW)            # [b, c, p, k]
    d = d.transpose(2, 1, 0, 3)                   # [p, c, b, k]
    d = d.reshape(CHUNK, Ct, R, GROUPS, CW)       # [p, c, r, g, k]  (b = 8r+g)
    d = d.transpose(0, 3, 1, 2, 4)                # [p, g, c, r, k]
    return np.ascontiguousarray(d.reshape(CHUNK, GROUPS, Ct, GW))


def _op_np_dtype():
    return mybir.dt.np(OP_DT)


def shard_inputs(coords_input, coords_target, num_atoms, order, assign, core_chunks):
    import ml_dtypes

    B, ncols = coords_input.shape
    nmax = ncols // 3
    sel = np.zeros((GW, R * CW), np.float32)
    for j in range(R * CW):
        sel[j, j] = 1.0
    sel = sel.astype(ml_dtypes.bfloat16)
    in_maps = []
    core_row_idx = []
    for c in range(N_CORES):
        m = {"sel": sel}
        idx_all = []
        Kn = len(assign[c])
        meta = np.zeros((ROWS, Kn), np.float32)
        for t in range(Kn):
            gt = assign[c][t]
            idx = order[gt * ROWS : (gt + 1) * ROWS]
            idx_all.append(idx)
            na = num_atoms[idx]
            meta[:, t] = na.astype(np.float32)
            x = coords_input[idx].reshape(ROWS, nmax, 3)
            y = coords_target[idx].reshape(ROWS, nmax, 3)
            op = _pack_tile(x, y, na, core_chunks[c][t])
            m[f"op{t}"] = np.ascontiguousarray(
                op.reshape(CHUNK, -1)
            ).astype(_op_np_dtype())
        m["meta"] = meta
        in_maps.append(m)
        core_row_idx.append(np.concatenate(idx_all))
    return in_maps, core_row_idx


def unshard_outputs(results, core_row_idx, B):
    out = np.empty(B, dtype=np.float32)
    for c in range(N_CORES):
        o = results[c]["out"]  # [ROWS, K]
        out[core_row_idx[c]] = o.T.reshape(-1)
    return out


# ---------------------------------------------------------------------------
# Entry point
# ---------------------------------------------------------------------------
_PROG_CACHE = {}


def _get_program(chunks):
    key = tuple(chunks)
    if key not in _PROG_CACHE:
        _PROG_CACHE[key] = build_program(list(chunks))
    return _PROG_CACHE[key]


def kernel(coords_input, coords_target, num_atoms):
    from concourse.bass_utils import run_bass_kernel_spmd

    x = np.ascontiguousarray(np.asarray(coords_input, dtype=np.float32))
    y = np.ascontiguousarray(np.asarray(coords_target, dtype=np.float32))
    na = np.asarray(num_atoms).astype(np.int64)
    B, ncols = x.shape
    Kn = B // (N_CORES * ROWS)
    assert B == N_CORES * ROWS * Kn, f"unsupported batch {B}"

    order, assign, core_chunks = plan_shards(na, n_classes=Kn)
    in_maps, core_row_idx = shard_inputs(x, y, na, order, assign, core_chunks)
    # group cores by identical chunk tuples -> one program per group
    groups = {}
    for c in range(N_CORES):
        groups.setdefault(tuple(core_chunks[c]), []).append(c)
    results = [None] * N_CORES
    for chunks, cores in groups.items():
        nc = _get_program(chunks)
        res = run_bass_kernel_spmd(
            nc, [in_maps[c] for c in cores], core_ids=list(range(len(cores)))
        )
        for i, c in enumerate(cores):
            results[c] = res.results[i]
    out = unshard_outputs(results, core_row_idx, B)
    return out.astype(np.float32)
